# revision 35
# baseline (speedup 1.0000x reference)
"""Trainium2 Bass kernel for nn_MultiHeadDaubechiesBlock.

Data-parallel over batch B=8 across 8 NeuronCores (one sequence per core).
Per-core pipeline:
  LN1 (DVE stats / ACT sqrt / GPSIMD apply) interleaved with DWT Toeplitz
  matmuls (emission-order software pipelining keeps PE fed), then per
  512-token chunk: interp (banded matmuls, single PSUM accumulation group
  per feature tile) -> proj bf16 + residual -> LN2 -> FFN1/FFN2 in fp8
  e4m3 DoubleRow perf mode (2x PE throughput; weights host-scaled x16,
  rescaled at gelu / output evac) -> residual -> out.
Level-2 detail+approx interp sources are folded via (f0+f1) filter algebra
into a single source. LN gains/biases are folded into adjacent GEMM
weights host-side (exact). Wavelet filters assumed constant across
heads/channels (true for this module); values taken from h0/h1 at runtime.
"""
import numpy as np
import ml_dtypes

B, T, D, H, DH, LEVELS, FFN = 8, 4096, 512, 4, 128, 3, 2048
P = 128
NT = T // P          # 32 token tiles
NDT = D // P         # 4 feature tiles
NFT = FFN // P       # 16 ffn tiles
NCH = 8              # t-chunks of 512
NWS = [2047, 1023, 511]
LPADS = [4096, 2048, 1024]
NGS = [32, 16, 8]    # dwt groups per level (64 windows each)
EPS = 1e-5
BF16 = ml_dtypes.bfloat16
FP8 = ml_dtypes.float8_e4m3
WS = 16.0            # fp8 weight pre-scale


# ----------------------------------------------------------------- plan
def _interp_mat(L, out_size=T):
    src = np.maximum((np.arange(out_size, dtype=np.float64) + 0.5) * (L / out_size) - 0.5, 0.0)
    i0 = np.clip(np.floor(src).astype(np.int64), 0, L - 1)
    i1 = np.minimum(i0 + 1, L - 1)
    w = src - i0
    U = np.zeros((out_size, L), np.float64)
    U[np.arange(out_size), i0] += 1.0 - w
    U[np.arange(out_size), i1] += w
    return U.astype(np.float32)


def make_plan():
    """Input-value-independent schedule + interp weight blocks."""
    # interp: 3 sources (lvl2 detail+approx folded): per chunk (s, kt, K, idx)
    Us = [_interp_mat(L) for L in NWS]
    ublks = []
    isched = [[] for _ in range(NCH)]
    for c in range(NCH):
        for s in range(3):
            U, L = Us[s], NWS[s]
            cols = U[512 * c:512 * (c + 1)]
            nz = np.nonzero(cols.any(0))[0]
            for kt in range(nz.min() // P, nz.max() // P + 1):
                K = min(P, L - P * kt)
                blk = cols[:, P * kt:P * kt + K].T
                if not np.any(blk):
                    continue
                full = np.zeros((P, 512), np.float32)
                full[:K] = blk
                isched[c].append((s, kt, K, len(ublks)))
                ublks.append(full)
    ublk = np.stack(ublks)

    # chunk readiness: required dwt group count per level
    creq = []
    for c in range(NCH):
        req = [0, 0, 0]
        for (s, kt, K, idx) in isched[c]:
            g = (P * kt + K - 1) // 64
            req[s] = max(req[s], g + 1)
        creq.append(req)

    # unified emission schedule: ('ln1', i) / ('dwt', lvl, g, last) /
    # ('A', c) [interp+proj+LN2 chain] / ('B', c) [transpose+FFN+out].
    # Two-stage pipeline: B(c) emitted after A(c+1) so the PE has work
    # while chunk c's LN2 stat chain completes on DVE/ACT.
    sched = []
    em = [0, 0, 0]
    cdone = 0

    def dwt_ready(lvl, g):
        last = g == NGS[lvl] - 1
        if lvl == 0:
            # LN1 applies: per-tile below tile 12, then batches of 4
            applied = em_ln1 if em_ln1 <= 12 else 12 + ((em_ln1 - 12) // 4) * 4
            return (g + (0 if last else 1)) <= applied - 1
        need = 2 * g + (2 if last else 3)
        return need <= em[lvl - 1]

    em_ln1 = 0
    for i in range(NT):
        sched.append(('ln1', i))
        em_ln1 += 1
        moved = True
        while moved:
            moved = False
            for lvl in range(LEVELS):
                while em[lvl] < NGS[lvl] and dwt_ready(lvl, em[lvl]):
                    g = em[lvl]
                    sched.append(('dwt', lvl, g, g == NGS[lvl] - 1))
                    em[lvl] += 1
                    moved = True
            # pace chunks vs LN1 so per-engine queues stay interleaved
            while (cdone < NCH and all(creq[cdone][l] <= em[l] for l in range(3))
                   and em_ln1 >= min(12 + 3 * cdone, NT)):
                sched.append(('A', cdone))
                if cdone >= 1:
                    sched.append(('B', cdone - 1))
                cdone += 1
                moved = True
    while cdone < NCH:
        sched.append(('A', cdone))
        if cdone >= 1:
            sched.append(('B', cdone - 1))
        cdone += 1
    sched.append(('B', NCH - 1))
    return {"isched": isched, "ublk": ublk, "nb": len(ublks), "sched": sched}


def _toeplitz(nw, Lp, f):
    F = np.zeros((nw, Lp), np.float32)
    for w in range(nw):
        F[w, 2 * w:2 * w + 4] = f
    return F


def make_flags(inputs):
    """Input-structure flags: skip device work for all-zero biases."""
    ln1_b = np.asarray(inputs["ln1_b"], np.float32)
    proj_b = np.asarray(inputs["proj_b"], np.float32)
    b2 = np.asarray(inputs["b2"], np.float32)
    return (bool(np.any(ln1_b) or np.any(proj_b)), bool(np.any(b2)))


def make_consts(inputs, plan):
    """Host-side constants (depend on input values)."""
    h0, h1 = np.asarray(inputs["h0"]), np.asarray(inputs["h1"])
    f0 = h0[:, 0, :, 0].astype(np.float32)
    f1 = h1[:, 0, :, 0].astype(np.float32)
    ln1_g = np.asarray(inputs["ln1_g"], np.float32)
    ln1_b = np.asarray(inputs["ln1_b"], np.float32)
    ln2_g = np.asarray(inputs["ln2_g"], np.float32)
    ln2_b = np.asarray(inputs["ln2_b"], np.float32)
    proj_w = np.asarray(inputs["proj_w"], np.float32)
    proj_b = np.asarray(inputs["proj_b"], np.float32)
    w1 = np.asarray(inputs["w1"], np.float32)
    b1 = np.asarray(inputs["b1"], np.float32)
    w2 = np.asarray(inputs["w2"], np.float32)
    b2 = np.asarray(inputs["b2"], np.float32)

    # DWT lhsT blocks [9,128,128]:
    #  lvl 0/1: merged cols 0..63 low (f0), 64..127 high (f1)
    #  lvl 2:   cols 0..63 combined filter (f0+f1)  [d2 + a3 fold]
    fblk = np.zeros((9, P, P), np.float32)
    for lvl in range(LEVELS):
        A = fblk[lvl * 3 + 0]
        for r in range(P):
            for w in range(64):
                k = r - 2 * w
                if 0 <= k < 4:
                    if lvl < 2:
                        A[r, w] = f0[lvl][k]
                        A[r, 64 + w] = f1[lvl][k]
                    else:
                        A[r, w] = f0[lvl][k] + f1[lvl][k]
        Bt = fblk[lvl * 3 + 1]
        for r in range(2):
            if lvl < 2:
                Bt[r, 63] = f0[lvl][r + 2]
                Bt[r, 127] = f1[lvl][r + 2]
            else:
                Bt[r, 63] = f0[lvl][r + 2] + f1[lvl][r + 2]
        Al = fblk[lvl * 3 + 2]
        Al[:] = A
        Al[:, 63] = 0.0
        if lvl < 2:
            Al[:, 127] = 0.0

    # m1 = wavelet operator applied to ones(T) (for ln1_b fold)
    ones = np.ones((T, 1), np.float32)
    a = ones
    comb = np.zeros((T, 1), np.float32)
    Us = [_interp_mat(L) for L in [NWS[0], NWS[1], NWS[2], NWS[2]]]
    for lvl in range(LEVELS):
        ap = np.zeros((LPADS[lvl], 1), np.float32)
        ap[:a.shape[0]] = a
        comb += Us[lvl] @ (_toeplitz(NWS[lvl], LPADS[lvl], f1[lvl]) @ ap)
        a = _toeplitz(NWS[lvl], LPADS[lvl], f0[lvl]) @ ap
    comb += Us[3] @ a
    m1 = comb[:, 0]

    wg = (ln1_g[:, None] * proj_w)                # LN1 g fold
    bW = ln1_b @ proj_w                           # LN1 b fold (rank-1 with m1)
    w1g = (ln2_g[:, None] * w1)                   # LN2 g fold
    b1f = b1 + ln2_b @ w1                         # LN2 b fold

    return {
        "wg": wg.astype(BF16),
        "w1": np.asarray(w1g * WS, FP8),
        "w2": np.asarray(w2 * WS, FP8),
        "fblk": fblk.reshape(9 * P, P).astype(BF16),
        "ublk": plan["ublk"].reshape(-1, 512).astype(BF16),
        "b1c": np.ascontiguousarray(b1f.reshape(NFT, P).T.astype(np.float32)),
        "r1l": np.stack([np.ones(T, np.float32), m1]).astype(BF16),
        "r1r": np.stack([proj_b, bW]).astype(BF16),
        "b2r": (b2 * WS).reshape(1, D).astype(BF16),
        "idn": np.identity(P, np.float32).astype(BF16),
    }


# ----------------------------------------------------------------- bass
def build_nc(plan, flags=(True, True)):
    need_r1, need_b2 = flags
    import concourse.bacc as bacc
    import concourse.tile as tile
    from concourse import mybir

    F32, BF, F8 = mybir.dt.float32, mybir.dt.bfloat16, mybir.dt.float8e4
    AF = mybir.ActivationFunctionType
    OP = mybir.AluOpType
    DR = mybir.MatmulPerfMode.DoubleRow
    INV = 1.0 / WS

    nc = bacc.Bacc("TRN2", target_bir_lowering=False, debug=False, name="daub")
    x_d = nc.dram_tensor("x", [T, D], F32, kind="ExternalInput")
    out_d = nc.dram_tensor("out", [T, D], F32, kind="ExternalOutput")
    wg_d = nc.dram_tensor("wg", [D, D], BF, kind="ExternalInput")
    w1_d = nc.dram_tensor("w1", [D, FFN], F8, kind="ExternalInput")
    w2_d = nc.dram_tensor("w2", [FFN, D], F8, kind="ExternalInput")
    fblk_d = nc.dram_tensor("fblk", [9 * P, P], BF, kind="ExternalInput")
    ublk_d = nc.dram_tensor("ublk", [plan["nb"] * P, 512], BF, kind="ExternalInput")
    b1c_d = nc.dram_tensor("b1c", [P, NFT], F32, kind="ExternalInput")
    r1l_d = nc.dram_tensor("r1l", [2, T], BF, kind="ExternalInput")
    r1r_d = nc.dram_tensor("r1r", [2, D], BF, kind="ExternalInput")
    b2r_d = nc.dram_tensor("b2r", [1, D], BF, kind="ExternalInput")
    idn_d = nc.dram_tensor("idn", [P, P], BF, kind="ExternalInput")

    with tile.TileContext(nc) as tc:
        import contextlib
        ctx = contextlib.ExitStack()
        pw = ctx.enter_context(tc.tile_pool(name="pw", bufs=1))
        pbig = ctx.enter_context(tc.tile_pool(name="pbig", bufs=1))
        pio = ctx.enter_context(tc.tile_pool(name="pio", bufs=4))
        pu = ctx.enter_context(tc.tile_pool(name="pu", bufs=20))
        px2 = ctx.enter_context(tc.tile_pool(name="px2", bufs=8))
        pcomb = ctx.enter_context(tc.tile_pool(name="pcomb", bufs=2))
        pxn2 = ctx.enter_context(tc.tile_pool(name="pxn2", bufs=2))
        ptm = ctx.enter_context(tc.tile_pool(name="ptm", bufs=8))
        ps_i = ctx.enter_context(tc.tile_pool(name="ps_i", bufs=2, space="PSUM"))
        ps_p = ctx.enter_context(tc.tile_pool(name="ps_p", bufs=2, space="PSUM"))
        ps_h = ctx.enter_context(tc.tile_pool(name="ps_h", bufs=2, space="PSUM"))

        # ---- x tiles 0..3 + fblk first (split across both hwdge queues):
        # the LN1/DWT lead-in is gated on these; weights and proj/FFN consts
        # are issued in small parts interleaved with the x stream below.
        qs = [nc.sync, nc.scalar]
        xts_pre = []
        for i in range(4):
            xt = pio.tile([P, D], F32, tag="xt", bufs=6, name=f"xt{i}")
            qs[i % 2].dma_start(out=xt, in_=x_d[P * i:P * (i + 1), :])
            xts_pre.append(xt)
        fblk_sb = pw.tile([P, 9, P], BF, name="fblk_sb")
        nc.scalar.dma_start(out=fblk_sb, in_=fblk_d.rearrange("(b p) m -> p b m", p=P))
        eps_sb = pw.tile([P, 1], F32, name="eps_sb")
        nc.vector.memset(eps_sb, EPS)
        # pre-warm the Sqrt ACT table while x tile 0 is still in flight
        warm_sb = pw.tile([P, 1], F32, name="warm_sb")
        nc.scalar.activation(out=warm_sb, in_=eps_sb, func=AF.Sqrt, bias=eps_sb)
        idn_sb = pw.tile([P, P], BF, name="idn_sb")
        b1c_sb = pw.tile([P, NFT], F32, name="b1c_sb")
        r1l_sb = pw.tile([2, T], BF, name="r1l_sb")
        r1r_sb = pw.tile([2, D], BF, name="r1r_sb")
        b2r_sb = pw.tile([1, D], BF, name="b2r_sb")
        wg_sb = pw.tile([P, NDT, D], BF, name="wg_sb")
        w1_sb = pw.tile([P, NDT, FFN], F8, name="w1_sb")
        w2_sb = pw.tile([P, NFT, D], F8, name="w2_sb")

        # weight/const part-DMAs: (emit_at_ln1_tile, fn)
        wparts = []

        def _wpart(sb, dram, kt0, nkt):
            def go(q):
                q.dma_start(
                    out=sb[:, kt0:kt0 + nkt, :],
                    in_=dram[P * kt0:P * (kt0 + nkt), :].rearrange(
                        "(kt p) n -> p kt n", p=P))
            return go

        for kt in range(NDT):
            wparts.append(_wpart(wg_sb, wg_d, kt, 1))
        for kt in range(NDT):
            wparts.append(_wpart(w1_sb, w1_d, kt, 1))
        for kt in range(0, NFT, 4):
            wparts.append(_wpart(w2_sb, w2_d, kt, 4))

        def _small_consts(q):
            q.dma_start(out=idn_sb, in_=idn_d[:, :])
            q.dma_start(out=b1c_sb, in_=b1c_d[:, :])
            if need_r1 or need_b2:
                q.dma_start(out=r1l_sb, in_=r1l_d[:, :])
            if need_r1:
                q.dma_start(out=r1r_sb, in_=r1r_d[:, :])
            if need_b2:
                q.dma_start(out=b2r_sb, in_=b2r_d[:, :])
        wparts.append(_small_consts)

        # ---- big activations
        xh_sb = pbig.tile([P, NT, D], BF, name="xh_sb")     # xn
        a1 = pbig.tile([P, 16, D], BF, name="a1")           # lvl0 low; d1 -> [0:8]
        a2 = pbig.tile([P, 8, D], BF, name="a2")            # lvl1 low; d23 -> [0:4]
        d0 = pbig.tile([P, 16, D], BF, name="d0")
        hdn = pbig.tile([P, 2 * NFT, D], F8, name="hdn")
        mv1_sb = pbig.tile([P, NT, 2], F32, name="mv1_sb")  # LN1 stats kept to
        sd1_sb = pbig.tile([P, NT, 1], F32, name="sd1_sb")  # reconstruct x later
        # zero pad rows (2047th window of lvl0, 1023rd of lvl1)
        nc.vector.memset(a1[96:128, 15, :], 0.0)
        nc.vector.memset(a2[96:128, 7, :], 0.0)

        ubtiles = {}

        ln1_xts = {}

        def emit_ln1(i):
            if i < 4:
                xt = xts_pre[i]
            else:
                xt = pio.tile([P, D], F32, tag="xt", bufs=6, name=f"xt{i}")
                qs[i % 2].dma_start(out=xt, in_=x_d[P * i:P * (i + 1), :])
            if 4 <= i and i - 4 < len(wparts):
                wparts[i - 4](qs[(i + 1) % 2])
            ln1_xts[i] = xt
            st = pio.tile([P, 6], F32, tag="st", name=f"st{i}")
            nc.vector.bn_stats(out=st, in_=xt)
            nc.vector.bn_aggr(out=mv1_sb[:, i, :], in_=st)
            # tiles 0..11: per-tile chain (tight lead-in pipeline; no gelus
            # running yet so individual Sqrts cost no table reloads).
            # tiles 12+: batch 4 sqrts per Sqrt-table visit.
            def _apply(j, n):
                nc.scalar.activation(
                    out=sd1_sb[:, j:j + n, :], in_=mv1_sb[:, j:j + n, 1:2],
                    func=AF.Sqrt, bias=eps_sb)
                rsn = pio.tile([P, n, 1], F32, tag="rs", name=f"rs{j}")
                nc.vector.reciprocal(out=rsn, in_=sd1_sb[:, j:j + n, :])
                for k in range(n):
                    nc.vector.tensor_scalar(
                        out=xh_sb[:, j + k, :], in0=ln1_xts.pop(j + k),
                        scalar1=mv1_sb[:, j + k, 0:1], scalar2=rsn[:, k, :],
                        op0=OP.subtract, op1=OP.mult)
            if i < 12:
                _apply(i, 1)
            elif i % 4 == 3:
                _apply(i - 3, 4)

        srcs = [(xh_sb, 0), (a1, 0), (a2, 0)]
        lows = [(a1, 0), (a2, 0), (a2, 0)]   # lvl2 "low" = d23 fold
        highs = [(d0, 0), (a1, 0), (a2, 0)]

        def emit_dwt(lvl, g, last):
            (src, sb), (low, lb), (high, hb) = srcs[lvl], lows[lvl], highs[lvl]
            pst_ = ps_i.tile([P, D], F32, tag="int", name=f"dw{lvl}_{g}")
            nc.tensor.matmul(
                pst_, fblk_sb[:, lvl * 3 + (2 if last else 0), :],
                src[:, sb + g, :], start=True, stop=last)
            if not last:
                nc.tensor.matmul(
                    pst_, fblk_sb[:2, lvl * 3 + 1, :], src[:2, sb + g + 1, :],
                    start=False, stop=True)
            Mg = 63 if last else 64
            lo = 64 * (g % 2)
            if lvl == 2:
                nc.vector.tensor_copy(out=low[lo:lo + Mg, lb + g // 2, :],
                                      in_=pst_[0:Mg, :])
            else:
                nc.scalar.copy(out=low[lo:lo + Mg, lb + g // 2, :], in_=pst_[0:Mg, :])
                nc.vector.tensor_copy(out=high[lo:lo + Mg, hb + g // 2, :],
                                      in_=pst_[64:64 + Mg, :])

        dsrc = [(d0, 0), (a1, 0), (a2, 0)]
        cstate = {}

        def emit_stageA(c):
            for (s, kt, K, idx) in plan["isched"][c]:
                ut = pu.tile([P, 512], BF, tag="ub", name=f"ub{idx}")
                nc.sync.dma_start(out=ut, in_=ublk_d[P * idx:P * (idx + 1), :])
                ubtiles[idx] = ut
            # interp: one accumulation group per feature tile
            comb_c = pcomb.tile([P, NDT, 512], BF, tag="comb", name=f"comb{c}")
            sch = plan["isched"][c]
            for dt in range(NDT):
                psi = ps_i.tile([P, 512], F32, tag="int", name=f"pi{c}_{dt}")
                for q, (s, kt, K, idx) in enumerate(sch):
                    dt_, db_ = dsrc[s]
                    nc.tensor.matmul(
                        psi, dt_[:K, db_ + kt, P * dt:P * (dt + 1)],
                        ubtiles[idx][:K, :],
                        start=(q == 0), stop=(q == len(sch) - 1))
                nc.vector.tensor_copy(out=comb_c[:, dt, :], in_=psi)
            # proj (bf16) + residual + LN2 stat chain (completes during next
            # stage-A's PE work; stage B consumes tmts)
            x2ts, tmts = [], []
            mv24 = pio.tile([P, 4, 2], F32, tag="mv24", name=f"mv24_{c}")
            for tj in range(4):
                ti = 4 * c + tj
                psp = ps_p.tile([P, D], F32, tag="pp", name=f"pp{ti}")
                for dt in range(NDT):
                    nc.tensor.matmul(
                        psp, comb_c[:, dt, P * tj:P * (tj + 1)], wg_sb[:, dt, :],
                        start=(dt == 0), stop=(not need_r1 and dt == NDT - 1))
                if need_r1:
                    nc.tensor.matmul(
                        psp, r1l_sb[:, P * ti:P * (ti + 1)], r1r_sb[:, :],
                        start=False, stop=True)
                # reconstruct x = xn*sd + mu from resident bf16 xn + LN1 stats
                # (saves the 8MB DRAM re-read of x)
                xrec = pio.tile([P, D], F32, tag="xrec", name=f"xrec{ti}")
                nc.vector.tensor_scalar(
                    out=xrec, in0=xh_sb[:, ti, :], scalar1=sd1_sb[:, ti, :],
                    scalar2=mv1_sb[:, ti, 0:1], op0=OP.mult, op1=OP.add)
                x2t = px2.tile([P, D], F32, tag="x2t", name=f"x2t{ti}")
                nc.vector.tensor_tensor(out=x2t, in0=psp, in1=xrec, op=OP.add)
                x2ts.append(x2t)
                st = pio.tile([P, 6], F32, tag="st2", name=f"st2_{ti}")
                nc.vector.bn_stats(out=st, in_=x2t)
                nc.vector.bn_aggr(out=mv24[:, tj, :], in_=st)
            # batched LN2 sqrt/recip: one Sqrt-table visit per chunk
            sd24 = pio.tile([P, 4, 1], F32, tag="sd24", name=f"sd24_{c}")
            nc.scalar.activation(out=sd24, in_=mv24[:, :, 1:2],
                                 func=AF.Sqrt, bias=eps_sb)
            rs24 = pio.tile([P, 4, 1], F32, tag="rs24", name=f"rs24_{c}")
            nc.vector.reciprocal(out=rs24, in_=sd24)
            for tj in range(4):
                tmt = ptm.tile([P, D], BF, tag="tmt", name=f"tmt{4 * c + tj}")
                nc.vector.tensor_scalar(
                    out=tmt, in0=x2ts[tj], scalar1=mv24[:, tj, 0:1],
                    scalar2=rs24[:, tj, :], op0=OP.subtract, op1=OP.mult)
                tmts.append(tmt)
            cstate[c] = (x2ts, tmts)

        def emit_stageB(c):
            x2ts, tmts = cstate.pop(c)
            # transpose (bf16) -> xn2 feature-major, cast fp8 at evac.
            # tj-major order: tj0..2 transposes run while tj3's LN2 chain
            # finishes on DVE/ACT.
            xn2f = pxn2.tile([P, NDT, 512], F8, tag="xn2f", name=f"xn2f{c}")
            pstps = [ps_p.tile([P, 2, 512], BF, tag="pt", name=f"pt{c}_{dp}")
                     for dp in range(2)]
            for tj in range(4):
                for dt in range(NDT):
                    nc.tensor.transpose(
                        pstps[dt // 2][:, dt % 2, P * tj:P * (tj + 1)],
                        tmts[tj][:, P * dt:P * (dt + 1)], idn_sb)
            nc.scalar.copy(out=xn2f[:, 0:2, :], in_=pstps[0])
            nc.vector.tensor_copy(out=xn2f[:, 2:4, :], in_=pstps[1])
            # FFN1 fp8 DoubleRow + gelu (psum scaled by WS; rescale in ACT)
            hb = NFT * (c % 2)
            for ft in range(NFT):
                psh = ps_h.tile([P, 512], F32, tag="ph", name=f"ph{c}_{ft}")
                for q in range(2):
                    nc.tensor.matmul(
                        psh, w1_sb[:, 2 * q:2 * q + 2, P * ft:P * (ft + 1)],
                        xn2f[:, 2 * q:2 * q + 2, :],
                        start=(q == 0), stop=(q == 1), perf_mode=DR)
                nc.scalar.activation(
                    out=hdn[:, hb + ft, :], in_=psh, func=AF.Gelu,
                    bias=b1c_sb[:, ft:ft + 1], scale=INV)
            # FFN2 fp8 DoubleRow + residual -> out (reverse accumulation so
            # the last matmul depends on the earliest-finished gelus)
            for tj in range(4):
                ti = 4 * c + tj
                pso = ps_p.tile([P, D], F32, tag="pp", name=f"po{ti}")
                if need_b2:
                    nc.tensor.matmul(
                        pso, r1l_sb[0:1, P * ti:P * (ti + 1)], b2r_sb[:, :],
                        start=True, stop=False)
                for q in reversed(range(NFT // 2)):
                    nc.tensor.matmul(
                        pso, hdn[:, hb + 2 * q:hb + 2 * q + 2, P * tj:P * (tj + 1)],
                        w2_sb[:, 2 * q:2 * q + 2, :],
                        start=(not need_b2 and q == NFT // 2 - 1),
                        stop=(q == 0), perf_mode=DR)
                ot = pio.tile([P, D], F32, tag="ot", name=f"ot{ti}")
                nc.vector.scalar_tensor_tensor(
                    out=ot, in0=pso, scalar=INV, in1=x2ts[tj],
                    op0=OP.mult, op1=OP.add)
                # scalar hwdge queue is idle during the chunk phase
                nc.scalar.dma_start(out=out_d[P * ti:P * (ti + 1), :], in_=ot)

        for ev in plan["sched"]:
            if ev[0] == 'ln1':
                emit_ln1(ev[1])
            elif ev[0] == 'dwt':
                emit_dwt(ev[1], ev[2], ev[3])
            elif ev[0] == 'A':
                emit_stageA(ev[1])
            else:
                emit_stageB(ev[1])
        ctx.close()
    nc.compile()
    return nc


_BUILT = {}


def _get_built(flags):
    if _BUILT.get("flags") != flags:
        plan = make_plan()
        _BUILT["plan"] = plan
        _BUILT["nc"] = build_nc(plan, flags)
        _BUILT["flags"] = flags
    return _BUILT["nc"], _BUILT["plan"]


def kernel(**inputs):
    from concourse.bass_utils import run_bass_kernel_spmd

    nc, plan = _get_built(make_flags(inputs))
    consts = make_consts(inputs, plan)
    x = np.ascontiguousarray(np.asarray(inputs["x"], np.float32))
    in_maps = []
    for b in range(B):
        m = {"x": np.ascontiguousarray(x[b])}
        m.update(consts)
        in_maps.append(m)
    res = run_bass_kernel_spmd(nc, in_maps, core_ids=list(range(B)))
    out = np.stack([res.results[b]["out"] for b in range(B)]).astype(np.float32)
    return out


# revision 38
# speedup vs baseline: 1.0042x; 1.0042x over previous
"""Trainium2 Bass kernel for nn_MultiHeadDaubechiesBlock.

Data-parallel over batch B=8 across 8 NeuronCores (one sequence per core).
Per-core pipeline:
  LN1 (DVE stats / ACT sqrt / GPSIMD apply) interleaved with DWT Toeplitz
  matmuls (emission-order software pipelining keeps PE fed), then per
  512-token chunk: interp (banded matmuls, single PSUM accumulation group
  per feature tile) -> proj bf16 + residual -> LN2 -> FFN1/FFN2 in fp8
  e4m3 DoubleRow perf mode (2x PE throughput; weights host-scaled x16,
  rescaled at gelu / output evac) -> residual -> out.
Level-2 detail+approx interp sources are folded via (f0+f1) filter algebra
into a single source. LN gains/biases are folded into adjacent GEMM
weights host-side (exact). Wavelet filters assumed constant across
heads/channels (true for this module); values taken from h0/h1 at runtime.
"""
import numpy as np
import ml_dtypes

B, T, D, H, DH, LEVELS, FFN = 8, 4096, 512, 4, 128, 3, 2048
P = 128
NT = T // P          # 32 token tiles
NDT = D // P         # 4 feature tiles
NFT = FFN // P       # 16 ffn tiles
NCH = 8              # t-chunks of 512
NWS = [2047, 1023, 511]
LPADS = [4096, 2048, 1024]
NGS = [32, 16, 8]    # dwt groups per level (64 windows each)
EPS = 1e-5
BF16 = ml_dtypes.bfloat16
FP8 = ml_dtypes.float8_e4m3
WS = 16.0            # fp8 weight pre-scale


# ----------------------------------------------------------------- plan
def _interp_mat(L, out_size=T):
    src = np.maximum((np.arange(out_size, dtype=np.float64) + 0.5) * (L / out_size) - 0.5, 0.0)
    i0 = np.clip(np.floor(src).astype(np.int64), 0, L - 1)
    i1 = np.minimum(i0 + 1, L - 1)
    w = src - i0
    U = np.zeros((out_size, L), np.float64)
    U[np.arange(out_size), i0] += 1.0 - w
    U[np.arange(out_size), i1] += w
    return U.astype(np.float32)


def make_plan():
    """Input-value-independent schedule + interp weight blocks."""
    # interp: 3 sources (lvl2 detail+approx folded): per chunk (s, kt, K, idx)
    Us = [_interp_mat(L) for L in NWS]
    ublks = []
    isched = [[] for _ in range(NCH)]
    for c in range(NCH):
        for s in range(3):
            U, L = Us[s], NWS[s]
            cols = U[512 * c:512 * (c + 1)]
            nz = np.nonzero(cols.any(0))[0]
            for kt in range(nz.min() // P, nz.max() // P + 1):
                K = min(P, L - P * kt)
                blk = cols[:, P * kt:P * kt + K].T
                if not np.any(blk):
                    continue
                full = np.zeros((P, 512), np.float32)
                full[:K] = blk
                isched[c].append((s, kt, K, len(ublks)))
                ublks.append(full)
    ublk = np.stack(ublks)

    # chunk readiness: required dwt group count per level
    creq = []
    for c in range(NCH):
        req = [0, 0, 0]
        for (s, kt, K, idx) in isched[c]:
            g = (P * kt + K - 1) // 64
            req[s] = max(req[s], g + 1)
        creq.append(req)

    # unified emission schedule: ('ln1', i) / ('dwt', lvl, g, last) /
    # ('A', c) [interp+proj+LN2 chain] / ('B', c) [transpose+FFN+out].
    # Two-stage pipeline: B(c) emitted after A(c+1) so the PE has work
    # while chunk c's LN2 stat chain completes on DVE/ACT.
    sched = []
    em = [0, 0, 0]
    cdone = 0

    def dwt_ready(lvl, g):
        last = g == NGS[lvl] - 1
        if lvl == 0:
            # LN1 applies: per-tile below tile 12, then batches of 4
            applied = em_ln1 if em_ln1 <= 12 else 12 + ((em_ln1 - 12) // 4) * 4
            return (g + (0 if last else 1)) <= applied - 1
        need = 2 * g + (2 if last else 3)
        return need <= em[lvl - 1]

    em_ln1 = 0
    for i in range(NT):
        sched.append(('ln1', i))
        em_ln1 += 1
        moved = True
        while moved:
            moved = False
            for lvl in range(LEVELS):
                while em[lvl] < NGS[lvl] and dwt_ready(lvl, em[lvl]):
                    g = em[lvl]
                    sched.append(('dwt', lvl, g, g == NGS[lvl] - 1))
                    em[lvl] += 1
                    moved = True
            # pace chunks vs LN1 so per-engine queues stay interleaved
            while (cdone < NCH and all(creq[cdone][l] <= em[l] for l in range(3))
                   and em_ln1 >= min(12 + 3 * cdone, NT)):
                sched.append(('A', cdone))
                if cdone >= 1:
                    sched.append(('B', cdone - 1))
                cdone += 1
                moved = True
    while cdone < NCH:
        sched.append(('A', cdone))
        if cdone >= 1:
            sched.append(('B', cdone - 1))
        cdone += 1
    sched.append(('B', NCH - 1))
    return {"isched": isched, "ublk": ublk, "nb": len(ublks), "sched": sched}


def _toeplitz(nw, Lp, f):
    F = np.zeros((nw, Lp), np.float32)
    for w in range(nw):
        F[w, 2 * w:2 * w + 4] = f
    return F


def make_flags(inputs):
    """Input-structure flags: skip device work for all-zero biases."""
    ln1_b = np.asarray(inputs["ln1_b"], np.float32)
    proj_b = np.asarray(inputs["proj_b"], np.float32)
    b2 = np.asarray(inputs["b2"], np.float32)
    return (bool(np.any(ln1_b) or np.any(proj_b)), bool(np.any(b2)))


def make_consts(inputs, plan):
    """Host-side constants (depend on input values)."""
    h0, h1 = np.asarray(inputs["h0"]), np.asarray(inputs["h1"])
    f0 = h0[:, 0, :, 0].astype(np.float32)
    f1 = h1[:, 0, :, 0].astype(np.float32)
    ln1_g = np.asarray(inputs["ln1_g"], np.float32)
    ln1_b = np.asarray(inputs["ln1_b"], np.float32)
    ln2_g = np.asarray(inputs["ln2_g"], np.float32)
    ln2_b = np.asarray(inputs["ln2_b"], np.float32)
    proj_w = np.asarray(inputs["proj_w"], np.float32)
    proj_b = np.asarray(inputs["proj_b"], np.float32)
    w1 = np.asarray(inputs["w1"], np.float32)
    b1 = np.asarray(inputs["b1"], np.float32)
    w2 = np.asarray(inputs["w2"], np.float32)
    b2 = np.asarray(inputs["b2"], np.float32)

    # DWT lhsT blocks [9,128,128]:
    #  lvl 0/1: merged cols 0..63 low (f0), 64..127 high (f1)
    #  lvl 2:   cols 0..63 combined filter (f0+f1)  [d2 + a3 fold]
    fblk = np.zeros((9, P, P), np.float32)
    for lvl in range(LEVELS):
        A = fblk[lvl * 3 + 0]
        for r in range(P):
            for w in range(64):
                k = r - 2 * w
                if 0 <= k < 4:
                    if lvl < 2:
                        A[r, w] = f0[lvl][k]
                        A[r, 64 + w] = f1[lvl][k]
                    else:
                        A[r, w] = f0[lvl][k] + f1[lvl][k]
        Bt = fblk[lvl * 3 + 1]
        for r in range(2):
            if lvl < 2:
                Bt[r, 63] = f0[lvl][r + 2]
                Bt[r, 127] = f1[lvl][r + 2]
            else:
                Bt[r, 63] = f0[lvl][r + 2] + f1[lvl][r + 2]
        Al = fblk[lvl * 3 + 2]
        Al[:] = A
        Al[:, 63] = 0.0
        if lvl < 2:
            Al[:, 127] = 0.0

    # m1 = wavelet operator applied to ones(T) (for ln1_b fold)
    ones = np.ones((T, 1), np.float32)
    a = ones
    comb = np.zeros((T, 1), np.float32)
    Us = [_interp_mat(L) for L in [NWS[0], NWS[1], NWS[2], NWS[2]]]
    for lvl in range(LEVELS):
        ap = np.zeros((LPADS[lvl], 1), np.float32)
        ap[:a.shape[0]] = a
        comb += Us[lvl] @ (_toeplitz(NWS[lvl], LPADS[lvl], f1[lvl]) @ ap)
        a = _toeplitz(NWS[lvl], LPADS[lvl], f0[lvl]) @ ap
    comb += Us[3] @ a
    m1 = comb[:, 0]

    wg = (ln1_g[:, None] * proj_w)                # LN1 g fold
    bW = ln1_b @ proj_w                           # LN1 b fold (rank-1 with m1)
    w1g = (ln2_g[:, None] * w1)                   # LN2 g fold
    b1f = b1 + ln2_b @ w1                         # LN2 b fold

    return {
        "wg": wg.astype(BF16),
        "w1": np.asarray(w1g * WS, FP8),
        "w2": np.asarray(w2 * WS, FP8),
        "fblk": fblk.reshape(9 * P, P).astype(BF16),
        "ublk": plan["ublk"].reshape(-1, 512).astype(BF16),
        "b1c": np.ascontiguousarray(b1f.reshape(NFT, P).T.astype(np.float32)),
        "r1l": np.stack([np.ones(T, np.float32), m1]).astype(BF16),
        "r1r": np.stack([proj_b, bW]).astype(BF16),
        "b2r": (b2 * WS).reshape(1, D).astype(BF16),
        "idn": np.identity(P, np.float32).astype(BF16),
    }


# ----------------------------------------------------------------- bass
def build_nc(plan, flags=(True, True)):
    need_r1, need_b2 = flags
    import concourse.bacc as bacc
    import concourse.tile as tile
    from concourse import mybir

    F32, BF, F8 = mybir.dt.float32, mybir.dt.bfloat16, mybir.dt.float8e4
    AF = mybir.ActivationFunctionType
    OP = mybir.AluOpType
    DR = mybir.MatmulPerfMode.DoubleRow
    INV = 1.0 / WS

    nc = bacc.Bacc("TRN2", target_bir_lowering=False, debug=False, name="daub")
    x_d = nc.dram_tensor("x", [T, D], F32, kind="ExternalInput")
    out_d = nc.dram_tensor("out", [T, D], F32, kind="ExternalOutput")
    wg_d = nc.dram_tensor("wg", [D, D], BF, kind="ExternalInput")
    w1_d = nc.dram_tensor("w1", [D, FFN], F8, kind="ExternalInput")
    w2_d = nc.dram_tensor("w2", [FFN, D], F8, kind="ExternalInput")
    fblk_d = nc.dram_tensor("fblk", [9 * P, P], BF, kind="ExternalInput")
    ublk_d = nc.dram_tensor("ublk", [plan["nb"] * P, 512], BF, kind="ExternalInput")
    b1c_d = nc.dram_tensor("b1c", [P, NFT], F32, kind="ExternalInput")
    r1l_d = nc.dram_tensor("r1l", [2, T], BF, kind="ExternalInput")
    r1r_d = nc.dram_tensor("r1r", [2, D], BF, kind="ExternalInput")
    b2r_d = nc.dram_tensor("b2r", [1, D], BF, kind="ExternalInput")
    idn_d = nc.dram_tensor("idn", [P, P], BF, kind="ExternalInput")

    with tile.TileContext(nc) as tc:
        import contextlib
        ctx = contextlib.ExitStack()
        pw = ctx.enter_context(tc.tile_pool(name="pw", bufs=1))
        pbig = ctx.enter_context(tc.tile_pool(name="pbig", bufs=1))
        pio = ctx.enter_context(tc.tile_pool(name="pio", bufs=4))
        pu = ctx.enter_context(tc.tile_pool(name="pu", bufs=20))
        px2 = ctx.enter_context(tc.tile_pool(name="px2", bufs=8))
        pcomb = ctx.enter_context(tc.tile_pool(name="pcomb", bufs=2))
        pxn2 = ctx.enter_context(tc.tile_pool(name="pxn2", bufs=2))
        ptm = ctx.enter_context(tc.tile_pool(name="ptm", bufs=8))
        ps_i = ctx.enter_context(tc.tile_pool(name="ps_i", bufs=2, space="PSUM"))
        ps_p = ctx.enter_context(tc.tile_pool(name="ps_p", bufs=2, space="PSUM"))
        ps_h = ctx.enter_context(tc.tile_pool(name="ps_h", bufs=2, space="PSUM"))

        # ---- x tiles 0..3 + fblk first (split across both hwdge queues):
        # the LN1/DWT lead-in is gated on these; weights and proj/FFN consts
        # are issued in small parts interleaved with the x stream below.
        qs = [nc.sync, nc.scalar]
        xts_pre = []
        for i in range(4):
            xt = pio.tile([P, D], F32, tag="xt", bufs=6, name=f"xt{i}")
            qs[i % 2].dma_start(out=xt, in_=x_d[P * i:P * (i + 1), :])
            xts_pre.append(xt)
        fblk_sb = pw.tile([P, 9, P], BF, name="fblk_sb")
        nc.scalar.dma_start(out=fblk_sb, in_=fblk_d.rearrange("(b p) m -> p b m", p=P))
        eps_sb = pw.tile([P, 1], F32, name="eps_sb")
        nc.vector.memset(eps_sb, EPS)
        # pre-warm the Sqrt ACT table while x tile 0 is still in flight
        warm_sb = pw.tile([P, 1], F32, name="warm_sb")
        nc.scalar.activation(out=warm_sb, in_=eps_sb, func=AF.Sqrt, bias=eps_sb)
        # PE pacer: filler matmuls keep the HAM clock gate ramping to 8/8
        # through the DMA/LN1-bound lead-in (zeros; result never read)
        flr_sb = pw.tile([P, P], BF, name="flr_sb")
        nc.vector.memset(flr_sb, 0.0)
        flr_ps = ps_h.tile([P, P], F32, tag="ph", name="flr_ps")
        flr_state = {"n": 0, "on": True}

        def emit_fillers(k):
            for _ in range(k):
                nc.tensor.matmul(flr_ps, flr_sb, flr_sb,
                                 start=(flr_state["n"] == 0), stop=False)
                flr_state["n"] += 1

        emit_fillers(40)
        idn_sb = pw.tile([P, P], BF, name="idn_sb")
        b1c_sb = pw.tile([P, NFT], F32, name="b1c_sb")
        r1l_sb = pw.tile([2, T], BF, name="r1l_sb")
        r1r_sb = pw.tile([2, D], BF, name="r1r_sb")
        b2r_sb = pw.tile([1, D], BF, name="b2r_sb")
        wg_sb = pw.tile([P, NDT, D], BF, name="wg_sb")
        w1_sb = pw.tile([P, NDT, FFN], F8, name="w1_sb")
        w2_sb = pw.tile([P, NFT, D], F8, name="w2_sb")

        # weight/const part-DMAs: (emit_at_ln1_tile, fn)
        wparts = []

        def _wpart(sb, dram, kt0, nkt):
            def go(q):
                q.dma_start(
                    out=sb[:, kt0:kt0 + nkt, :],
                    in_=dram[P * kt0:P * (kt0 + nkt), :].rearrange(
                        "(kt p) n -> p kt n", p=P))
            return go

        for kt in range(NDT):
            wparts.append(_wpart(wg_sb, wg_d, kt, 1))
        for kt in range(NDT):
            wparts.append(_wpart(w1_sb, w1_d, kt, 1))
        for kt in range(0, NFT, 4):
            wparts.append(_wpart(w2_sb, w2_d, kt, 4))

        def _small_consts(q):
            q.dma_start(out=idn_sb, in_=idn_d[:, :])
            q.dma_start(out=b1c_sb, in_=b1c_d[:, :])
            if need_r1 or need_b2:
                q.dma_start(out=r1l_sb, in_=r1l_d[:, :])
            if need_r1:
                q.dma_start(out=r1r_sb, in_=r1r_d[:, :])
            if need_b2:
                q.dma_start(out=b2r_sb, in_=b2r_d[:, :])
        wparts.append(_small_consts)

        # ---- big activations
        xh_sb = pbig.tile([P, NT, D], BF, name="xh_sb")     # xn
        a1 = pbig.tile([P, 16, D], BF, name="a1")           # lvl0 low; d1 -> [0:8]
        a2 = pbig.tile([P, 8, D], BF, name="a2")            # lvl1 low; d23 -> [0:4]
        d0 = pbig.tile([P, 16, D], BF, name="d0")
        hdn = pbig.tile([P, 2 * NFT, D], F8, name="hdn")
        mv1_sb = pbig.tile([P, NT, 2], F32, name="mv1_sb")  # LN1 stats kept to
        sd1_sb = pbig.tile([P, NT, 1], F32, name="sd1_sb")  # reconstruct x later
        # zero pad rows (2047th window of lvl0, 1023rd of lvl1)
        nc.vector.memset(a1[96:128, 15, :], 0.0)
        nc.vector.memset(a2[96:128, 7, :], 0.0)

        ubtiles = {}

        ln1_xts = {}

        def emit_ln1(i):
            if i < 4:
                xt = xts_pre[i]
            else:
                xt = pio.tile([P, D], F32, tag="xt", bufs=6, name=f"xt{i}")
                qs[i % 2].dma_start(out=xt, in_=x_d[P * i:P * (i + 1), :])
            if 4 <= i and i - 4 < len(wparts):
                wparts[i - 4](qs[(i + 1) % 2])
            ln1_xts[i] = xt
            st = pio.tile([P, 6], F32, tag="st", name=f"st{i}")
            nc.vector.bn_stats(out=st, in_=xt)
            nc.vector.bn_aggr(out=mv1_sb[:, i, :], in_=st)
            # tiles 0..11: per-tile chain (tight lead-in pipeline; no gelus
            # running yet so individual Sqrts cost no table reloads).
            # tiles 12+: batch 4 sqrts per Sqrt-table visit.
            def _apply(j, n):
                nc.scalar.activation(
                    out=sd1_sb[:, j:j + n, :], in_=mv1_sb[:, j:j + n, 1:2],
                    func=AF.Sqrt, bias=eps_sb)
                rsn = pio.tile([P, n, 1], F32, tag="rs", name=f"rs{j}")
                nc.vector.reciprocal(out=rsn, in_=sd1_sb[:, j:j + n, :])
                for k in range(n):
                    nc.vector.tensor_scalar(
                        out=xh_sb[:, j + k, :], in0=ln1_xts.pop(j + k),
                        scalar1=mv1_sb[:, j + k, 0:1], scalar2=rsn[:, k, :],
                        op0=OP.subtract, op1=OP.mult)
            if i < 12:
                _apply(i, 1)
            elif i % 4 == 3:
                _apply(i - 3, 4)

        srcs = [(xh_sb, 0), (a1, 0), (a2, 0)]
        lows = [(a1, 0), (a2, 0), (a2, 0)]   # lvl2 "low" = d23 fold
        highs = [(d0, 0), (a1, 0), (a2, 0)]

        def emit_dwt(lvl, g, last):
            (src, sb), (low, lb), (high, hb) = srcs[lvl], lows[lvl], highs[lvl]
            pst_ = ps_i.tile([P, D], F32, tag="int", name=f"dw{lvl}_{g}")
            nc.tensor.matmul(
                pst_, fblk_sb[:, lvl * 3 + (2 if last else 0), :],
                src[:, sb + g, :], start=True, stop=last)
            if not last:
                nc.tensor.matmul(
                    pst_, fblk_sb[:2, lvl * 3 + 1, :], src[:2, sb + g + 1, :],
                    start=False, stop=True)
            Mg = 63 if last else 64
            lo = 64 * (g % 2)
            if lvl == 2:
                nc.vector.tensor_copy(out=low[lo:lo + Mg, lb + g // 2, :],
                                      in_=pst_[0:Mg, :])
            else:
                nc.scalar.copy(out=low[lo:lo + Mg, lb + g // 2, :], in_=pst_[0:Mg, :])
                nc.vector.tensor_copy(out=high[lo:lo + Mg, hb + g // 2, :],
                                      in_=pst_[64:64 + Mg, :])

        dsrc = [(d0, 0), (a1, 0), (a2, 0)]
        cstate = {}

        def emit_stageA(c):
            for (s, kt, K, idx) in plan["isched"][c]:
                ut = pu.tile([P, 512], BF, tag="ub", name=f"ub{idx}")
                nc.sync.dma_start(out=ut, in_=ublk_d[P * idx:P * (idx + 1), :])
                ubtiles[idx] = ut
            # interp: one accumulation group per feature tile
            comb_c = pcomb.tile([P, NDT, 512], BF, tag="comb", name=f"comb{c}")
            sch = plan["isched"][c]
            for dt in range(NDT):
                psi = ps_i.tile([P, 512], F32, tag="int", name=f"pi{c}_{dt}")
                for q, (s, kt, K, idx) in enumerate(sch):
                    dt_, db_ = dsrc[s]
                    nc.tensor.matmul(
                        psi, dt_[:K, db_ + kt, P * dt:P * (dt + 1)],
                        ubtiles[idx][:K, :],
                        start=(q == 0), stop=(q == len(sch) - 1))
                nc.vector.tensor_copy(out=comb_c[:, dt, :], in_=psi)
            # proj (bf16) + residual + LN2 stat chain (completes during next
            # stage-A's PE work; stage B consumes tmts)
            x2ts, tmts = [], []
            mv24 = pio.tile([P, 4, 2], F32, tag="mv24", name=f"mv24_{c}")
            for tj in range(4):
                ti = 4 * c + tj
                psp = ps_p.tile([P, D], F32, tag="pp", name=f"pp{ti}")
                for dt in range(NDT):
                    nc.tensor.matmul(
                        psp, comb_c[:, dt, P * tj:P * (tj + 1)], wg_sb[:, dt, :],
                        start=(dt == 0), stop=(not need_r1 and dt == NDT - 1))
                if need_r1:
                    nc.tensor.matmul(
                        psp, r1l_sb[:, P * ti:P * (ti + 1)], r1r_sb[:, :],
                        start=False, stop=True)
                # reconstruct x = xn*sd + mu from resident bf16 xn + LN1 stats
                # (saves the 8MB DRAM re-read of x)
                xrec = pio.tile([P, D], F32, tag="xrec", name=f"xrec{ti}")
                nc.vector.tensor_scalar(
                    out=xrec, in0=xh_sb[:, ti, :], scalar1=sd1_sb[:, ti, :],
                    scalar2=mv1_sb[:, ti, 0:1], op0=OP.mult, op1=OP.add)
                x2t = px2.tile([P, D], F32, tag="x2t", name=f"x2t{ti}")
                nc.vector.tensor_tensor(out=x2t, in0=psp, in1=xrec, op=OP.add)
                x2ts.append(x2t)
                st = pio.tile([P, 6], F32, tag="st2", name=f"st2_{ti}")
                nc.vector.bn_stats(out=st, in_=x2t)
                nc.vector.bn_aggr(out=mv24[:, tj, :], in_=st)
            # batched LN2 sqrt/recip: one Sqrt-table visit per chunk
            sd24 = pio.tile([P, 4, 1], F32, tag="sd24", name=f"sd24_{c}")
            nc.scalar.activation(out=sd24, in_=mv24[:, :, 1:2],
                                 func=AF.Sqrt, bias=eps_sb)
            rs24 = pio.tile([P, 4, 1], F32, tag="rs24", name=f"rs24_{c}")
            nc.vector.reciprocal(out=rs24, in_=sd24)
            for tj in range(4):
                tmt = ptm.tile([P, D], BF, tag="tmt", name=f"tmt{4 * c + tj}")
                nc.vector.tensor_scalar(
                    out=tmt, in0=x2ts[tj], scalar1=mv24[:, tj, 0:1],
                    scalar2=rs24[:, tj, :], op0=OP.subtract, op1=OP.mult)
                tmts.append(tmt)
            cstate[c] = (x2ts, tmts)

        def emit_stageB(c):
            x2ts, tmts = cstate.pop(c)
            # transpose (bf16) -> xn2 feature-major, cast fp8 at evac.
            # tj-major order: tj0..2 transposes run while tj3's LN2 chain
            # finishes on DVE/ACT.
            xn2f = pxn2.tile([P, NDT, 512], F8, tag="xn2f", name=f"xn2f{c}")
            pstps = [ps_p.tile([P, 2, 512], BF, tag="pt", name=f"pt{c}_{dp}")
                     for dp in range(2)]
            for tj in range(4):
                for dt in range(NDT):
                    nc.tensor.transpose(
                        pstps[dt // 2][:, dt % 2, P * tj:P * (tj + 1)],
                        tmts[tj][:, P * dt:P * (dt + 1)], idn_sb)
            nc.scalar.copy(out=xn2f[:, 0:2, :], in_=pstps[0])
            nc.vector.tensor_copy(out=xn2f[:, 2:4, :], in_=pstps[1])
            # FFN1 fp8 DoubleRow + gelu (psum scaled by WS; rescale in ACT)
            hb = NFT * (c % 2)
            for ft in range(NFT):
                psh = ps_h.tile([P, 512], F32, tag="ph", name=f"ph{c}_{ft}")
                for q in range(2):
                    nc.tensor.matmul(
                        psh, w1_sb[:, 2 * q:2 * q + 2, P * ft:P * (ft + 1)],
                        xn2f[:, 2 * q:2 * q + 2, :],
                        start=(q == 0), stop=(q == 1), perf_mode=DR)
                nc.scalar.activation(
                    out=hdn[:, hb + ft, :], in_=psh, func=AF.Gelu,
                    bias=b1c_sb[:, ft:ft + 1], scale=INV)
            # FFN2 fp8 DoubleRow + residual -> out (reverse accumulation so
            # the last matmul depends on the earliest-finished gelus)
            for tj in range(4):
                ti = 4 * c + tj
                pso = ps_p.tile([P, D], F32, tag="pp", name=f"po{ti}")
                if need_b2:
                    nc.tensor.matmul(
                        pso, r1l_sb[0:1, P * ti:P * (ti + 1)], b2r_sb[:, :],
                        start=True, stop=False)
                for q in reversed(range(NFT // 2)):
                    nc.tensor.matmul(
                        pso, hdn[:, hb + 2 * q:hb + 2 * q + 2, P * tj:P * (tj + 1)],
                        w2_sb[:, 2 * q:2 * q + 2, :],
                        start=(not need_b2 and q == NFT // 2 - 1),
                        stop=(q == 0), perf_mode=DR)
                ot = pio.tile([P, D], F32, tag="ot", name=f"ot{ti}")
                nc.vector.scalar_tensor_tensor(
                    out=ot, in0=pso, scalar=INV, in1=x2ts[tj],
                    op0=OP.mult, op1=OP.add)
                nc.sync.dma_start(out=out_d[P * ti:P * (ti + 1), :], in_=ot)

        for ev in plan["sched"]:
            if ev[0] == 'ln1':
                emit_ln1(ev[1])
                if flr_state["on"]:
                    emit_fillers(4)
            elif ev[0] == 'dwt':
                emit_dwt(ev[1], ev[2], ev[3])
            elif ev[0] == 'A':
                emit_stageA(ev[1])
            else:
                if flr_state["on"]:
                    # close the filler accumulation group before FFN1 reuses
                    # the ph psum ring
                    nc.tensor.matmul(flr_ps, flr_sb, flr_sb,
                                     start=False, stop=True)
                    flr_state["on"] = False
                emit_stageB(ev[1])
        ctx.close()
    nc.compile()
    return nc


_BUILT = {}


def _get_built(flags):
    if _BUILT.get("flags") != flags:
        plan = make_plan()
        _BUILT["plan"] = plan
        _BUILT["nc"] = build_nc(plan, flags)
        _BUILT["flags"] = flags
    return _BUILT["nc"], _BUILT["plan"]


def kernel(**inputs):
    from concourse.bass_utils import run_bass_kernel_spmd

    nc, plan = _get_built(make_flags(inputs))
    consts = make_consts(inputs, plan)
    x = np.ascontiguousarray(np.asarray(inputs["x"], np.float32))
    in_maps = []
    for b in range(B):
        m = {"x": np.ascontiguousarray(x[b])}
        m.update(consts)
        in_maps.append(m)
    res = run_bass_kernel_spmd(nc, in_maps, core_ids=list(range(B)))
    out = np.stack([res.results[b]["out"] for b in range(B)]).astype(np.float32)
    return out


# revision 41
# speedup vs baseline: 1.0425x; 1.0381x over previous
"""Trainium2 Bass kernel for nn_MultiHeadDaubechiesBlock.

Data-parallel over batch B=8 across 8 NeuronCores (one sequence per core).
Per-core pipeline:
  LN1 (DVE stats / ACT sqrt / GPSIMD apply) interleaved with DWT Toeplitz
  matmuls (emission-order software pipelining keeps PE fed), then per
  512-token chunk: interp (banded matmuls, single PSUM accumulation group
  per feature tile) -> proj bf16 + residual -> LN2 -> FFN1/FFN2 in fp8
  e4m3 DoubleRow perf mode (2x PE throughput; weights host-scaled x16,
  rescaled at gelu / output evac) -> residual -> out.
Level-2 detail+approx interp sources are folded via (f0+f1) filter algebra
into a single source. LN gains/biases are folded into adjacent GEMM
weights host-side (exact). Wavelet filters assumed constant across
heads/channels (true for this module); values taken from h0/h1 at runtime.
"""
import numpy as np
import ml_dtypes

B, T, D, H, DH, LEVELS, FFN = 8, 4096, 512, 4, 128, 3, 2048
P = 128
NT = T // P          # 32 token tiles
NDT = D // P         # 4 feature tiles
NFT = FFN // P       # 16 ffn tiles
NCH = 8              # t-chunks of 512
NWS = [2047, 1023, 511]
LPADS = [4096, 2048, 1024]
NGS = [32, 16, 8]    # dwt groups per level (64 windows each)
EPS = 1e-5
BF16 = ml_dtypes.bfloat16
FP8 = ml_dtypes.float8_e4m3
WS = 16.0            # fp8 weight pre-scale


# ----------------------------------------------------------------- plan
def _interp_mat(L, out_size=T):
    src = np.maximum((np.arange(out_size, dtype=np.float64) + 0.5) * (L / out_size) - 0.5, 0.0)
    i0 = np.clip(np.floor(src).astype(np.int64), 0, L - 1)
    i1 = np.minimum(i0 + 1, L - 1)
    w = src - i0
    U = np.zeros((out_size, L), np.float64)
    U[np.arange(out_size), i0] += 1.0 - w
    U[np.arange(out_size), i1] += w
    return U.astype(np.float32)


def make_plan():
    """Input-value-independent schedule + interp weight blocks."""
    # interp: 3 sources (lvl2 detail+approx folded): per chunk (s, kt, K, idx)
    Us = [_interp_mat(L) for L in NWS]
    ublks = []
    isched = [[] for _ in range(NCH)]
    for c in range(NCH):
        for s in range(3):
            U, L = Us[s], NWS[s]
            cols = U[512 * c:512 * (c + 1)]
            nz = np.nonzero(cols.any(0))[0]
            for kt in range(nz.min() // P, nz.max() // P + 1):
                K = min(P, L - P * kt)
                blk = cols[:, P * kt:P * kt + K].T
                if not np.any(blk):
                    continue
                full = np.zeros((P, 512), np.float32)
                full[:K] = blk
                isched[c].append((s, kt, K, len(ublks)))
                ublks.append(full)
    ublk = np.stack(ublks)

    # chunk readiness: required dwt group count per level
    creq = []
    for c in range(NCH):
        req = [0, 0, 0]
        for (s, kt, K, idx) in isched[c]:
            g = (P * kt + K - 1) // 64
            req[s] = max(req[s], g + 1)
        creq.append(req)

    # unified emission schedule: ('ln1', i) / ('dwt', lvl, g, last) /
    # ('A', c) [interp+proj+LN2 chain] / ('B', c) [transpose+FFN+out].
    # Two-stage pipeline: B(c) emitted after A(c+1) so the PE has work
    # while chunk c's LN2 stat chain completes on DVE/ACT.
    sched = []
    em = [0, 0, 0]
    cdone = 0

    def dwt_ready(lvl, g):
        last = g == NGS[lvl] - 1
        if lvl == 0:
            # LN1 applies: per-tile below tile 12, then batches of 4
            applied = em_ln1 if em_ln1 <= 12 else 12 + ((em_ln1 - 12) // 4) * 4
            return (g + (0 if last else 1)) <= applied - 1
        need = 2 * g + (2 if last else 3)
        return need <= em[lvl - 1]

    em_ln1 = 0
    for i in range(NT):
        sched.append(('ln1', i))
        em_ln1 += 1
        moved = True
        while moved:
            moved = False
            for lvl in range(LEVELS):
                while em[lvl] < NGS[lvl] and dwt_ready(lvl, em[lvl]):
                    g = em[lvl]
                    sched.append(('dwt', lvl, g, g == NGS[lvl] - 1))
                    em[lvl] += 1
                    moved = True
            # pace chunks vs LN1 so per-engine queues stay interleaved
            while (cdone < NCH and all(creq[cdone][l] <= em[l] for l in range(3))
                   and em_ln1 >= min(12 + 3 * cdone, NT)):
                sched.append(('A', cdone))
                if cdone >= 1:
                    sched.append(('B', cdone - 1))
                cdone += 1
                moved = True
    while cdone < NCH:
        sched.append(('A', cdone))
        if cdone >= 1:
            sched.append(('B', cdone - 1))
        cdone += 1
    sched.append(('B', NCH - 1))
    return {"isched": isched, "ublk": ublk, "nb": len(ublks), "sched": sched}


def _toeplitz(nw, Lp, f):
    F = np.zeros((nw, Lp), np.float32)
    for w in range(nw):
        F[w, 2 * w:2 * w + 4] = f
    return F


def make_flags(inputs):
    """Input-structure flags: skip device work for all-zero biases."""
    ln1_b = np.asarray(inputs["ln1_b"], np.float32)
    proj_b = np.asarray(inputs["proj_b"], np.float32)
    b2 = np.asarray(inputs["b2"], np.float32)
    return (bool(np.any(ln1_b) or np.any(proj_b)), bool(np.any(b2)))


def make_consts(inputs, plan):
    """Host-side constants (depend on input values)."""
    h0, h1 = np.asarray(inputs["h0"]), np.asarray(inputs["h1"])
    f0 = h0[:, 0, :, 0].astype(np.float32)
    f1 = h1[:, 0, :, 0].astype(np.float32)
    ln1_g = np.asarray(inputs["ln1_g"], np.float32)
    ln1_b = np.asarray(inputs["ln1_b"], np.float32)
    ln2_g = np.asarray(inputs["ln2_g"], np.float32)
    ln2_b = np.asarray(inputs["ln2_b"], np.float32)
    proj_w = np.asarray(inputs["proj_w"], np.float32)
    proj_b = np.asarray(inputs["proj_b"], np.float32)
    w1 = np.asarray(inputs["w1"], np.float32)
    b1 = np.asarray(inputs["b1"], np.float32)
    w2 = np.asarray(inputs["w2"], np.float32)
    b2 = np.asarray(inputs["b2"], np.float32)

    # DWT lhsT blocks [9,128,128]:
    #  lvl 0/1: merged cols 0..63 low (f0), 64..127 high (f1)
    #  lvl 2:   cols 0..63 combined filter (f0+f1)  [d2 + a3 fold]
    fblk = np.zeros((9, P, P), np.float32)
    for lvl in range(LEVELS):
        A = fblk[lvl * 3 + 0]
        for r in range(P):
            for w in range(64):
                k = r - 2 * w
                if 0 <= k < 4:
                    if lvl < 2:
                        A[r, w] = f0[lvl][k]
                        A[r, 64 + w] = f1[lvl][k]
                    else:
                        A[r, w] = f0[lvl][k] + f1[lvl][k]
        Bt = fblk[lvl * 3 + 1]
        for r in range(2):
            if lvl < 2:
                Bt[r, 63] = f0[lvl][r + 2]
                Bt[r, 127] = f1[lvl][r + 2]
            else:
                Bt[r, 63] = f0[lvl][r + 2] + f1[lvl][r + 2]
        Al = fblk[lvl * 3 + 2]
        Al[:] = A
        Al[:, 63] = 0.0
        if lvl < 2:
            Al[:, 127] = 0.0

    # m1 = wavelet operator applied to ones(T) (for ln1_b fold)
    ones = np.ones((T, 1), np.float32)
    a = ones
    comb = np.zeros((T, 1), np.float32)
    Us = [_interp_mat(L) for L in [NWS[0], NWS[1], NWS[2], NWS[2]]]
    for lvl in range(LEVELS):
        ap = np.zeros((LPADS[lvl], 1), np.float32)
        ap[:a.shape[0]] = a
        comb += Us[lvl] @ (_toeplitz(NWS[lvl], LPADS[lvl], f1[lvl]) @ ap)
        a = _toeplitz(NWS[lvl], LPADS[lvl], f0[lvl]) @ ap
    comb += Us[3] @ a
    m1 = comb[:, 0]

    wg = (ln1_g[:, None] * proj_w)                # LN1 g fold
    bW = ln1_b @ proj_w                           # LN1 b fold (rank-1 with m1)
    w1g = (ln2_g[:, None] * w1)                   # LN2 g fold
    b1f = b1 + ln2_b @ w1                         # LN2 b fold

    return {
        "wg": wg.astype(BF16),
        "w1": np.asarray(w1g * WS, FP8),
        "w2": np.asarray(w2 * WS, FP8),
        "fblk": fblk.reshape(9 * P, P).astype(BF16),
        "ublk": plan["ublk"].reshape(-1, 512).astype(BF16),
        "b1c": np.ascontiguousarray(b1f.reshape(NFT, P).T.astype(np.float32)),
        "r1l": np.stack([np.ones(T, np.float32), m1]).astype(BF16),
        "r1r": np.stack([proj_b, bW]).astype(BF16),
        "b2r": (b2 * WS).reshape(1, D).astype(BF16),
        "idn": np.identity(P, np.float32).astype(BF16),
    }


# ----------------------------------------------------------------- bass
def build_nc(plan, flags=(True, True)):
    need_r1, need_b2 = flags
    import concourse.bacc as bacc
    import concourse.tile as tile
    from concourse import mybir

    F32, BF, F8 = mybir.dt.float32, mybir.dt.bfloat16, mybir.dt.float8e4
    AF = mybir.ActivationFunctionType
    OP = mybir.AluOpType
    DR = mybir.MatmulPerfMode.DoubleRow
    INV = 1.0 / WS

    nc = bacc.Bacc("TRN2", target_bir_lowering=False, debug=False, name="daub")
    x_d = nc.dram_tensor("x", [T, D], F32, kind="ExternalInput")
    out_d = nc.dram_tensor("out", [T, D], F32, kind="ExternalOutput")
    wg_d = nc.dram_tensor("wg", [D, D], BF, kind="ExternalInput")
    w1_d = nc.dram_tensor("w1", [D, FFN], F8, kind="ExternalInput")
    w2_d = nc.dram_tensor("w2", [FFN, D], F8, kind="ExternalInput")
    fblk_d = nc.dram_tensor("fblk", [9 * P, P], BF, kind="ExternalInput")
    ublk_d = nc.dram_tensor("ublk", [plan["nb"] * P, 512], BF, kind="ExternalInput")
    b1c_d = nc.dram_tensor("b1c", [P, NFT], F32, kind="ExternalInput")
    r1l_d = nc.dram_tensor("r1l", [2, T], BF, kind="ExternalInput")
    r1r_d = nc.dram_tensor("r1r", [2, D], BF, kind="ExternalInput")
    b2r_d = nc.dram_tensor("b2r", [1, D], BF, kind="ExternalInput")
    idn_d = nc.dram_tensor("idn", [P, P], BF, kind="ExternalInput")

    with tile.TileContext(nc) as tc:
        import contextlib
        ctx = contextlib.ExitStack()
        pw = ctx.enter_context(tc.tile_pool(name="pw", bufs=1))
        pbig = ctx.enter_context(tc.tile_pool(name="pbig", bufs=1))
        pio = ctx.enter_context(tc.tile_pool(name="pio", bufs=4))
        pu = ctx.enter_context(tc.tile_pool(name="pu", bufs=20))
        px2 = ctx.enter_context(tc.tile_pool(name="px2", bufs=8))
        pcomb = ctx.enter_context(tc.tile_pool(name="pcomb", bufs=2))
        pxn2 = ctx.enter_context(tc.tile_pool(name="pxn2", bufs=2))
        ptm = ctx.enter_context(tc.tile_pool(name="ptm", bufs=8))
        ps_i = ctx.enter_context(tc.tile_pool(name="ps_i", bufs=2, space="PSUM"))
        ps_p = ctx.enter_context(tc.tile_pool(name="ps_p", bufs=2, space="PSUM"))
        ps_h = ctx.enter_context(tc.tile_pool(name="ps_h", bufs=2, space="PSUM"))

        # ---- x tiles 0..3 + fblk first (split across both hwdge queues):
        # the LN1/DWT lead-in is gated on these; weights and proj/FFN consts
        # are issued in small parts interleaved with the x stream below.
        qs = [nc.sync, nc.scalar]
        xts_pre = []
        for i in range(4):
            xt = pio.tile([P, D], F32, tag="xt", bufs=6, name=f"xt{i}")
            qs[i % 2].dma_start(out=xt, in_=x_d[P * i:P * (i + 1), :])
            xts_pre.append(xt)
        fblk_sb = pw.tile([P, 9, P], BF, name="fblk_sb")
        nc.scalar.dma_start(out=fblk_sb, in_=fblk_d.rearrange("(b p) m -> p b m", p=P))
        eps_sb = pw.tile([P, 1], F32, name="eps_sb")
        nc.vector.memset(eps_sb, EPS)
        # pre-warm the Sqrt ACT table while x tile 0 is still in flight
        warm_sb = pw.tile([P, 1], F32, name="warm_sb")
        nc.scalar.activation(out=warm_sb, in_=eps_sb, func=AF.Sqrt, bias=eps_sb)
        # PE pacer: filler matmuls keep the HAM clock gate ramping to 8/8
        # through the DMA/LN1-bound lead-in (zeros; result never read)
        flr_sb = pw.tile([P, P], BF, name="flr_sb")
        nc.vector.memset(flr_sb, 0.0)
        flr_ps = ps_h.tile([P, P], F32, tag="ph", name="flr_ps")
        flr_state = {"n": 0, "on": True}

        def emit_fillers(k):
            for _ in range(k):
                nc.tensor.matmul(flr_ps, flr_sb, flr_sb,
                                 start=(flr_state["n"] == 0), stop=False)
                flr_state["n"] += 1

        emit_fillers(40)
        idn_sb = pw.tile([P, P], BF, name="idn_sb")
        b1c_sb = pw.tile([P, NFT], F32, name="b1c_sb")
        r1l_sb = pw.tile([2, T], BF, name="r1l_sb")
        r1r_sb = pw.tile([2, D], BF, name="r1r_sb")
        b2r_sb = pw.tile([1, D], BF, name="b2r_sb")
        wg_sb = pw.tile([P, NDT, D], BF, name="wg_sb")
        w1_sb = pw.tile([P, NDT, FFN], F8, name="w1_sb")
        w2_sb = pw.tile([P, NFT, D], F8, name="w2_sb")

        # weight/const part-DMAs: (emit_at_ln1_tile, fn)
        wparts = []

        def _wpart(sb, dram, kt0, nkt):
            def go(q):
                q.dma_start(
                    out=sb[:, kt0:kt0 + nkt, :],
                    in_=dram[P * kt0:P * (kt0 + nkt), :].rearrange(
                        "(kt p) n -> p kt n", p=P))
            return go

        for kt in range(NDT):
            wparts.append(_wpart(wg_sb, wg_d, kt, 1))
        for kt in range(NDT):
            wparts.append(_wpart(w1_sb, w1_d, kt, 1))
        for kt in range(0, NFT, 4):
            wparts.append(_wpart(w2_sb, w2_d, kt, 4))

        def _small_consts(q):
            q.dma_start(out=idn_sb, in_=idn_d[:, :])
            q.dma_start(out=b1c_sb, in_=b1c_d[:, :])
            if need_r1 or need_b2:
                q.dma_start(out=r1l_sb, in_=r1l_d[:, :])
            if need_r1:
                q.dma_start(out=r1r_sb, in_=r1r_d[:, :])
            if need_b2:
                q.dma_start(out=b2r_sb, in_=b2r_d[:, :])
        wparts.append(_small_consts)

        # ---- big activations
        xh_sb = pbig.tile([P, NT, D], BF, name="xh_sb")     # xn
        a1 = pbig.tile([P, 16, D], BF, name="a1")           # lvl0 low; d1 -> [0:8]
        a2 = pbig.tile([P, 8, D], BF, name="a2")            # lvl1 low; d23 -> [0:4]
        d0 = pbig.tile([P, 16, D], BF, name="d0")
        hdn = pbig.tile([P, 2 * NFT, D], F8, name="hdn")
        mv1_sb = pbig.tile([P, NT, 2], F32, name="mv1_sb")  # LN1 stats kept to
        sd1_sb = pbig.tile([P, NT, 1], F32, name="sd1_sb")  # reconstruct x later
        # zero pad rows (2047th window of lvl0, 1023rd of lvl1)
        nc.vector.memset(a1[96:128, 15, :], 0.0)
        nc.vector.memset(a2[96:128, 7, :], 0.0)

        ubtiles = {}

        ln1_xts = {}

        def emit_ln1(i):
            if i < 4:
                xt = xts_pre[i]
            else:
                xt = pio.tile([P, D], F32, tag="xt", bufs=6, name=f"xt{i}")
                qs[i % 2].dma_start(out=xt, in_=x_d[P * i:P * (i + 1), :])
            if 4 <= i and i - 4 < len(wparts):
                wparts[i - 4](qs[(i + 1) % 2])
            ln1_xts[i] = xt
            st = pio.tile([P, 6], F32, tag="st", name=f"st{i}")
            nc.vector.bn_stats(out=st, in_=xt)
            nc.vector.bn_aggr(out=mv1_sb[:, i, :], in_=st)
            # tiles 0..11: per-tile chain (tight lead-in pipeline; no gelus
            # running yet so individual Sqrts cost no table reloads).
            # tiles 12+: batch 4 sqrts per Sqrt-table visit.
            def _apply(j, n):
                nc.scalar.activation(
                    out=sd1_sb[:, j:j + n, :], in_=mv1_sb[:, j:j + n, 1:2],
                    func=AF.Sqrt, bias=eps_sb)
                rsn = pio.tile([P, n, 1], F32, tag="rs", name=f"rs{j}")
                nc.vector.reciprocal(out=rsn, in_=sd1_sb[:, j:j + n, :])
                for k in range(n):
                    nc.vector.tensor_scalar(
                        out=xh_sb[:, j + k, :], in0=ln1_xts.pop(j + k),
                        scalar1=mv1_sb[:, j + k, 0:1], scalar2=rsn[:, k, :],
                        op0=OP.subtract, op1=OP.mult)
            if i < 12:
                _apply(i, 1)
            elif i % 4 == 3:
                _apply(i - 3, 4)

        srcs = [(xh_sb, 0), (a1, 0), (a2, 0)]
        lows = [(a1, 0), (a2, 0), (a2, 0)]   # lvl2 "low" = d23 fold
        highs = [(d0, 0), (a1, 0), (a2, 0)]

        def emit_dwt(lvl, g, last):
            (src, sb), (low, lb), (high, hb) = srcs[lvl], lows[lvl], highs[lvl]
            pst_ = ps_i.tile([P, D], F32, tag="int", name=f"dw{lvl}_{g}")
            nc.tensor.matmul(
                pst_, fblk_sb[:, lvl * 3 + (2 if last else 0), :],
                src[:, sb + g, :], start=True, stop=last)
            if not last:
                nc.tensor.matmul(
                    pst_, fblk_sb[:2, lvl * 3 + 1, :], src[:2, sb + g + 1, :],
                    start=False, stop=True)
            Mg = 63 if last else 64
            lo = 64 * (g % 2)
            if lvl == 2:
                nc.scalar.copy(out=low[lo:lo + Mg, lb + g // 2, :],
                               in_=pst_[0:Mg, :])
            else:
                nc.scalar.copy(out=low[lo:lo + Mg, lb + g // 2, :], in_=pst_[0:Mg, :])
                nc.vector.tensor_copy(out=high[lo:lo + Mg, hb + g // 2, :],
                                      in_=pst_[64:64 + Mg, :])

        dsrc = [(d0, 0), (a1, 0), (a2, 0)]
        cstate = {}

        def emit_stageA(c):
            for (s, kt, K, idx) in plan["isched"][c]:
                ut = pu.tile([P, 512], BF, tag="ub", name=f"ub{idx}")
                nc.sync.dma_start(out=ut, in_=ublk_d[P * idx:P * (idx + 1), :])
                ubtiles[idx] = ut
            # interp: one accumulation group per feature tile
            comb_c = pcomb.tile([P, NDT, 512], BF, tag="comb", name=f"comb{c}")
            sch = plan["isched"][c]
            for dt in range(NDT):
                psi = ps_i.tile([P, 512], F32, tag="int", name=f"pi{c}_{dt}")
                for q, (s, kt, K, idx) in enumerate(sch):
                    dt_, db_ = dsrc[s]
                    nc.tensor.matmul(
                        psi, dt_[:K, db_ + kt, P * dt:P * (dt + 1)],
                        ubtiles[idx][:K, :],
                        start=(q == 0), stop=(q == len(sch) - 1))
                nc.vector.tensor_copy(out=comb_c[:, dt, :], in_=psi)
            # proj (bf16) + residual + LN2 stat chain (completes during next
            # stage-A's PE work; stage B consumes tmts)
            x2ts, tmts = [], []
            mv24 = pio.tile([P, 4, 2], F32, tag="mv24", name=f"mv24_{c}")
            for tj in range(4):
                ti = 4 * c + tj
                psp = ps_p.tile([P, D], F32, tag="pp", name=f"pp{ti}")
                for dt in range(NDT):
                    nc.tensor.matmul(
                        psp, comb_c[:, dt, P * tj:P * (tj + 1)], wg_sb[:, dt, :],
                        start=(dt == 0), stop=(not need_r1 and dt == NDT - 1))
                if need_r1:
                    nc.tensor.matmul(
                        psp, r1l_sb[:, P * ti:P * (ti + 1)], r1r_sb[:, :],
                        start=False, stop=True)
                # reconstruct x = xn*sd + mu from resident bf16 xn + LN1 stats
                # (saves the 8MB DRAM re-read of x); ACT: Identity(sd*xn + mu)
                xrec = pio.tile([P, D], F32, tag="xrec", name=f"xrec{ti}")
                nc.scalar.activation(
                    out=xrec, in_=xh_sb[:, ti, :], func=AF.Identity,
                    scale=sd1_sb[:, ti, :], bias=mv1_sb[:, ti, 0:1])
                x2t = px2.tile([P, D], F32, tag="x2t", name=f"x2t{ti}")
                nc.vector.tensor_tensor(out=x2t, in0=psp, in1=xrec, op=OP.add)
                x2ts.append(x2t)
                st = pio.tile([P, 6], F32, tag="st2", name=f"st2_{ti}")
                nc.vector.bn_stats(out=st, in_=x2t)
                nc.vector.bn_aggr(out=mv24[:, tj, :], in_=st)
            # batched LN2 sqrt/recip: one Sqrt-table visit per chunk
            sd24 = pio.tile([P, 4, 1], F32, tag="sd24", name=f"sd24_{c}")
            nc.scalar.activation(out=sd24, in_=mv24[:, :, 1:2],
                                 func=AF.Sqrt, bias=eps_sb)
            rs24 = pio.tile([P, 4, 1], F32, tag="rs24", name=f"rs24_{c}")
            nc.vector.reciprocal(out=rs24, in_=sd24)
            for tj in range(4):
                tmt = ptm.tile([P, D], BF, tag="tmt", name=f"tmt{4 * c + tj}")
                nc.vector.tensor_scalar(
                    out=tmt, in0=x2ts[tj], scalar1=mv24[:, tj, 0:1],
                    scalar2=rs24[:, tj, :], op0=OP.subtract, op1=OP.mult)
                tmts.append(tmt)
            cstate[c] = (x2ts, tmts)

        def emit_stageB(c):
            x2ts, tmts = cstate.pop(c)
            # transpose (bf16) -> xn2 feature-major, cast fp8 at evac.
            # tj-major order: tj0..2 transposes run while tj3's LN2 chain
            # finishes on DVE/ACT.
            xn2f = pxn2.tile([P, NDT, 512], F8, tag="xn2f", name=f"xn2f{c}")
            pstps = [ps_p.tile([P, 2, 512], BF, tag="pt", name=f"pt{c}_{dp}")
                     for dp in range(2)]
            for tj in range(4):
                for dt in range(NDT):
                    nc.tensor.transpose(
                        pstps[dt // 2][:, dt % 2, P * tj:P * (tj + 1)],
                        tmts[tj][:, P * dt:P * (dt + 1)], idn_sb)
            nc.scalar.copy(out=xn2f[:, 0:2, :], in_=pstps[0])
            nc.vector.tensor_copy(out=xn2f[:, 2:4, :], in_=pstps[1])
            # FFN1 fp8 DoubleRow + gelu (psum scaled by WS; rescale in ACT)
            hb = NFT * (c % 2)
            for ft in range(NFT):
                psh = ps_h.tile([P, 512], F32, tag="ph", name=f"ph{c}_{ft}")
                for q in range(2):
                    nc.tensor.matmul(
                        psh, w1_sb[:, 2 * q:2 * q + 2, P * ft:P * (ft + 1)],
                        xn2f[:, 2 * q:2 * q + 2, :],
                        start=(q == 0), stop=(q == 1), perf_mode=DR)
                nc.scalar.activation(
                    out=hdn[:, hb + ft, :], in_=psh, func=AF.Gelu,
                    bias=b1c_sb[:, ft:ft + 1], scale=INV)
            # FFN2 fp8 DoubleRow + residual -> out (reverse accumulation so
            # the last matmul depends on the earliest-finished gelus)
            for tj in range(4):
                ti = 4 * c + tj
                pso = ps_p.tile([P, D], F32, tag="pp", name=f"po{ti}")
                if need_b2:
                    nc.tensor.matmul(
                        pso, r1l_sb[0:1, P * ti:P * (ti + 1)], b2r_sb[:, :],
                        start=True, stop=False)
                for q in reversed(range(NFT // 2)):
                    nc.tensor.matmul(
                        pso, hdn[:, hb + 2 * q:hb + 2 * q + 2, P * tj:P * (tj + 1)],
                        w2_sb[:, 2 * q:2 * q + 2, :],
                        start=(not need_b2 and q == NFT // 2 - 1),
                        stop=(q == 0), perf_mode=DR)
                ot = pio.tile([P, D], F32, tag="ot", name=f"ot{ti}")
                nc.vector.scalar_tensor_tensor(
                    out=ot, in0=pso, scalar=INV, in1=x2ts[tj],
                    op0=OP.mult, op1=OP.add)
                nc.sync.dma_start(out=out_d[P * ti:P * (ti + 1), :], in_=ot)

        for ev in plan["sched"]:
            if ev[0] == 'ln1':
                emit_ln1(ev[1])
                if flr_state["on"]:
                    emit_fillers(8)
            elif ev[0] == 'dwt':
                emit_dwt(ev[1], ev[2], ev[3])
                if flr_state["on"]:
                    emit_fillers(3)
            elif ev[0] == 'A':
                emit_stageA(ev[1])
            else:
                if flr_state["on"]:
                    # close the filler accumulation group before FFN1 reuses
                    # the ph psum ring
                    nc.tensor.matmul(flr_ps, flr_sb, flr_sb,
                                     start=False, stop=True)
                    flr_state["on"] = False
                emit_stageB(ev[1])
        ctx.close()
    nc.compile()
    return nc


_BUILT = {}


def _get_built(flags):
    if _BUILT.get("flags") != flags:
        plan = make_plan()
        _BUILT["plan"] = plan
        _BUILT["nc"] = build_nc(plan, flags)
        _BUILT["flags"] = flags
    return _BUILT["nc"], _BUILT["plan"]


def kernel(**inputs):
    from concourse.bass_utils import run_bass_kernel_spmd

    nc, plan = _get_built(make_flags(inputs))
    consts = make_consts(inputs, plan)
    x = np.ascontiguousarray(np.asarray(inputs["x"], np.float32))
    in_maps = []
    for b in range(B):
        m = {"x": np.ascontiguousarray(x[b])}
        m.update(consts)
        in_maps.append(m)
    res = run_bass_kernel_spmd(nc, in_maps, core_ids=list(range(B)))
    out = np.stack([res.results[b]["out"] for b in range(B)]).astype(np.float32)
    return out


# revision 42
# speedup vs baseline: 1.0519x; 1.0090x over previous
"""Trainium2 Bass kernel for nn_MultiHeadDaubechiesBlock.

Data-parallel over batch B=8 across 8 NeuronCores (one sequence per core).
Per-core pipeline:
  LN1 (DVE stats / ACT sqrt / GPSIMD apply) interleaved with DWT Toeplitz
  matmuls (emission-order software pipelining keeps PE fed), then per
  512-token chunk: interp (banded matmuls, single PSUM accumulation group
  per feature tile) -> proj bf16 + residual -> LN2 -> FFN1/FFN2 in fp8
  e4m3 DoubleRow perf mode (2x PE throughput; weights host-scaled x16,
  rescaled at gelu / output evac) -> residual -> out.
Level-2 detail+approx interp sources are folded via (f0+f1) filter algebra
into a single source. LN gains/biases are folded into adjacent GEMM
weights host-side (exact). Wavelet filters assumed constant across
heads/channels (true for this module); values taken from h0/h1 at runtime.
"""
import numpy as np
import ml_dtypes

B, T, D, H, DH, LEVELS, FFN = 8, 4096, 512, 4, 128, 3, 2048
P = 128
NT = T // P          # 32 token tiles
NDT = D // P         # 4 feature tiles
NFT = FFN // P       # 16 ffn tiles
NCH = 8              # t-chunks of 512
NWS = [2047, 1023, 511]
LPADS = [4096, 2048, 1024]
NGS = [32, 16, 8]    # dwt groups per level (64 windows each)
EPS = 1e-5
BF16 = ml_dtypes.bfloat16
FP8 = ml_dtypes.float8_e4m3
WS = 16.0            # fp8 weight pre-scale


# ----------------------------------------------------------------- plan
def _interp_mat(L, out_size=T):
    src = np.maximum((np.arange(out_size, dtype=np.float64) + 0.5) * (L / out_size) - 0.5, 0.0)
    i0 = np.clip(np.floor(src).astype(np.int64), 0, L - 1)
    i1 = np.minimum(i0 + 1, L - 1)
    w = src - i0
    U = np.zeros((out_size, L), np.float64)
    U[np.arange(out_size), i0] += 1.0 - w
    U[np.arange(out_size), i1] += w
    return U.astype(np.float32)


def make_plan():
    """Input-value-independent schedule + interp weight blocks."""
    # interp: 3 sources (lvl2 detail+approx folded): per chunk (s, kt, K, idx)
    Us = [_interp_mat(L) for L in NWS]
    ublks = []
    isched = [[] for _ in range(NCH)]
    for c in range(NCH):
        for s in range(3):
            U, L = Us[s], NWS[s]
            cols = U[512 * c:512 * (c + 1)]
            nz = np.nonzero(cols.any(0))[0]
            for kt in range(nz.min() // P, nz.max() // P + 1):
                K = min(P, L - P * kt)
                blk = cols[:, P * kt:P * kt + K].T
                if not np.any(blk):
                    continue
                full = np.zeros((P, 512), np.float32)
                full[:K] = blk
                isched[c].append((s, kt, K, len(ublks)))
                ublks.append(full)
    ublk = np.stack(ublks)

    # chunk readiness: required dwt group count per level
    creq = []
    for c in range(NCH):
        req = [0, 0, 0]
        for (s, kt, K, idx) in isched[c]:
            g = (P * kt + K - 1) // 64
            req[s] = max(req[s], g + 1)
        creq.append(req)

    # unified emission schedule: ('ln1', i) / ('dwt', lvl, g, last) /
    # ('A', c) [interp+proj+LN2 chain] / ('B', c) [transpose+FFN+out].
    # Two-stage pipeline: B(c) emitted after A(c+1) so the PE has work
    # while chunk c's LN2 stat chain completes on DVE/ACT.
    sched = []
    em = [0, 0, 0]
    cdone = 0

    def dwt_ready(lvl, g):
        last = g == NGS[lvl] - 1
        if lvl == 0:
            # LN1 applies: per-tile below tile 12, then batches of 4
            applied = em_ln1 if em_ln1 <= 12 else 12 + ((em_ln1 - 12) // 4) * 4
            return (g + (0 if last else 1)) <= applied - 1
        need = 2 * g + (2 if last else 3)
        return need <= em[lvl - 1]

    em_ln1 = 0
    for i in range(NT):
        sched.append(('ln1', i))
        em_ln1 += 1
        moved = True
        while moved:
            moved = False
            for lvl in range(LEVELS):
                while em[lvl] < NGS[lvl] and dwt_ready(lvl, em[lvl]):
                    g = em[lvl]
                    sched.append(('dwt', lvl, g, g == NGS[lvl] - 1))
                    em[lvl] += 1
                    moved = True
            # pace chunks vs LN1 so per-engine queues stay interleaved
            while (cdone < NCH and all(creq[cdone][l] <= em[l] for l in range(3))
                   and em_ln1 >= min(12 + 3 * cdone, NT)):
                sched.append(('A', cdone))
                if cdone >= 1:
                    sched.append(('B', cdone - 1))
                cdone += 1
                moved = True
    while cdone < NCH:
        sched.append(('A', cdone))
        if cdone >= 1:
            sched.append(('B', cdone - 1))
        cdone += 1
    sched.append(('B', NCH - 1))
    return {"isched": isched, "ublk": ublk, "nb": len(ublks), "sched": sched}


def _toeplitz(nw, Lp, f):
    F = np.zeros((nw, Lp), np.float32)
    for w in range(nw):
        F[w, 2 * w:2 * w + 4] = f
    return F


def make_flags(inputs):
    """Input-structure flags: skip device work for all-zero biases."""
    ln1_b = np.asarray(inputs["ln1_b"], np.float32)
    proj_b = np.asarray(inputs["proj_b"], np.float32)
    b2 = np.asarray(inputs["b2"], np.float32)
    return (bool(np.any(ln1_b) or np.any(proj_b)), bool(np.any(b2)))


def make_consts(inputs, plan):
    """Host-side constants (depend on input values)."""
    h0, h1 = np.asarray(inputs["h0"]), np.asarray(inputs["h1"])
    f0 = h0[:, 0, :, 0].astype(np.float32)
    f1 = h1[:, 0, :, 0].astype(np.float32)
    ln1_g = np.asarray(inputs["ln1_g"], np.float32)
    ln1_b = np.asarray(inputs["ln1_b"], np.float32)
    ln2_g = np.asarray(inputs["ln2_g"], np.float32)
    ln2_b = np.asarray(inputs["ln2_b"], np.float32)
    proj_w = np.asarray(inputs["proj_w"], np.float32)
    proj_b = np.asarray(inputs["proj_b"], np.float32)
    w1 = np.asarray(inputs["w1"], np.float32)
    b1 = np.asarray(inputs["b1"], np.float32)
    w2 = np.asarray(inputs["w2"], np.float32)
    b2 = np.asarray(inputs["b2"], np.float32)

    # DWT lhsT blocks [9,128,128]:
    #  lvl 0/1: merged cols 0..63 low (f0), 64..127 high (f1)
    #  lvl 2:   cols 0..63 combined filter (f0+f1)  [d2 + a3 fold]
    fblk = np.zeros((9, P, P), np.float32)
    for lvl in range(LEVELS):
        A = fblk[lvl * 3 + 0]
        for r in range(P):
            for w in range(64):
                k = r - 2 * w
                if 0 <= k < 4:
                    if lvl < 2:
                        A[r, w] = f0[lvl][k]
                        A[r, 64 + w] = f1[lvl][k]
                    else:
                        A[r, w] = f0[lvl][k] + f1[lvl][k]
        Bt = fblk[lvl * 3 + 1]
        for r in range(2):
            if lvl < 2:
                Bt[r, 63] = f0[lvl][r + 2]
                Bt[r, 127] = f1[lvl][r + 2]
            else:
                Bt[r, 63] = f0[lvl][r + 2] + f1[lvl][r + 2]
        Al = fblk[lvl * 3 + 2]
        Al[:] = A
        Al[:, 63] = 0.0
        if lvl < 2:
            Al[:, 127] = 0.0

    # m1 = wavelet operator applied to ones(T) (for ln1_b fold)
    ones = np.ones((T, 1), np.float32)
    a = ones
    comb = np.zeros((T, 1), np.float32)
    Us = [_interp_mat(L) for L in [NWS[0], NWS[1], NWS[2], NWS[2]]]
    for lvl in range(LEVELS):
        ap = np.zeros((LPADS[lvl], 1), np.float32)
        ap[:a.shape[0]] = a
        comb += Us[lvl] @ (_toeplitz(NWS[lvl], LPADS[lvl], f1[lvl]) @ ap)
        a = _toeplitz(NWS[lvl], LPADS[lvl], f0[lvl]) @ ap
    comb += Us[3] @ a
    m1 = comb[:, 0]

    wg = (ln1_g[:, None] * proj_w)                # LN1 g fold
    bW = ln1_b @ proj_w                           # LN1 b fold (rank-1 with m1)
    w1g = (ln2_g[:, None] * w1)                   # LN2 g fold
    b1f = b1 + ln2_b @ w1                         # LN2 b fold

    return {
        "wg": wg.astype(BF16),
        "w1": np.asarray(w1g * WS, FP8),
        "w2": np.asarray(w2 * WS, FP8),
        "fblk": fblk.reshape(9 * P, P).astype(BF16),
        "ublk": plan["ublk"].reshape(-1, 512).astype(BF16),
        "b1c": np.ascontiguousarray(b1f.reshape(NFT, P).T.astype(np.float32)),
        "r1l": np.stack([np.ones(T, np.float32), m1]).astype(BF16),
        "r1r": np.stack([proj_b, bW]).astype(BF16),
        "b2r": (b2 * WS).reshape(1, D).astype(BF16),
        "idn": np.identity(P, np.float32).astype(BF16),
    }


# ----------------------------------------------------------------- bass
def build_nc(plan, flags=(True, True)):
    need_r1, need_b2 = flags
    import concourse.bacc as bacc
    import concourse.tile as tile
    from concourse import mybir

    F32, BF, F8 = mybir.dt.float32, mybir.dt.bfloat16, mybir.dt.float8e4
    AF = mybir.ActivationFunctionType
    OP = mybir.AluOpType
    DR = mybir.MatmulPerfMode.DoubleRow
    INV = 1.0 / WS

    nc = bacc.Bacc("TRN2", target_bir_lowering=False, debug=False, name="daub")
    x_d = nc.dram_tensor("x", [T, D], F32, kind="ExternalInput")
    out_d = nc.dram_tensor("out", [T, D], F32, kind="ExternalOutput")
    wg_d = nc.dram_tensor("wg", [D, D], BF, kind="ExternalInput")
    w1_d = nc.dram_tensor("w1", [D, FFN], F8, kind="ExternalInput")
    w2_d = nc.dram_tensor("w2", [FFN, D], F8, kind="ExternalInput")
    fblk_d = nc.dram_tensor("fblk", [9 * P, P], BF, kind="ExternalInput")
    ublk_d = nc.dram_tensor("ublk", [plan["nb"] * P, 512], BF, kind="ExternalInput")
    b1c_d = nc.dram_tensor("b1c", [P, NFT], F32, kind="ExternalInput")
    r1l_d = nc.dram_tensor("r1l", [2, T], BF, kind="ExternalInput")
    r1r_d = nc.dram_tensor("r1r", [2, D], BF, kind="ExternalInput")
    b2r_d = nc.dram_tensor("b2r", [1, D], BF, kind="ExternalInput")
    idn_d = nc.dram_tensor("idn", [P, P], BF, kind="ExternalInput")

    with tile.TileContext(nc) as tc:
        import contextlib
        ctx = contextlib.ExitStack()
        pw = ctx.enter_context(tc.tile_pool(name="pw", bufs=1))
        pbig = ctx.enter_context(tc.tile_pool(name="pbig", bufs=1))
        pio = ctx.enter_context(tc.tile_pool(name="pio", bufs=4))
        pu = ctx.enter_context(tc.tile_pool(name="pu", bufs=20))
        px2 = ctx.enter_context(tc.tile_pool(name="px2", bufs=8))
        pcomb = ctx.enter_context(tc.tile_pool(name="pcomb", bufs=2))
        pxn2 = ctx.enter_context(tc.tile_pool(name="pxn2", bufs=2))
        ptm = ctx.enter_context(tc.tile_pool(name="ptm", bufs=8))
        ps_i = ctx.enter_context(tc.tile_pool(name="ps_i", bufs=2, space="PSUM"))
        ps_p = ctx.enter_context(tc.tile_pool(name="ps_p", bufs=2, space="PSUM"))
        ps_h = ctx.enter_context(tc.tile_pool(name="ps_h", bufs=2, space="PSUM"))

        # ---- x tiles 0..3 + fblk first (split across both hwdge queues):
        # the LN1/DWT lead-in is gated on these; weights and proj/FFN consts
        # are issued in small parts interleaved with the x stream below.
        qs = [nc.sync, nc.scalar]
        xts_pre = []
        for i in range(4):
            xt = pio.tile([P, D], F32, tag="xt", bufs=6, name=f"xt{i}")
            qs[i % 2].dma_start(out=xt, in_=x_d[P * i:P * (i + 1), :])
            xts_pre.append(xt)
        fblk_sb = pw.tile([P, 9, P], BF, name="fblk_sb")
        nc.scalar.dma_start(out=fblk_sb, in_=fblk_d.rearrange("(b p) m -> p b m", p=P))
        eps_sb = pw.tile([P, 1], F32, name="eps_sb")
        nc.vector.memset(eps_sb, EPS)
        # pre-warm the Sqrt ACT table while x tile 0 is still in flight
        warm_sb = pw.tile([P, 1], F32, name="warm_sb")
        nc.scalar.activation(out=warm_sb, in_=eps_sb, func=AF.Sqrt, bias=eps_sb)
        # PE pacer: filler matmuls keep the HAM clock gate ramping to 8/8
        # through the DMA/LN1-bound lead-in (zeros; result never read)
        flr_sb = pw.tile([P, P], BF, name="flr_sb")
        nc.vector.memset(flr_sb, 0.0)
        flr_ps = ps_h.tile([P, P], F32, tag="ph", name="flr_ps")
        flr_state = {"n": 0, "on": True}

        def emit_fillers(k):
            for _ in range(k):
                nc.tensor.matmul(flr_ps, flr_sb, flr_sb,
                                 start=(flr_state["n"] == 0), stop=False)
                flr_state["n"] += 1

        emit_fillers(40)
        idn_sb = pw.tile([P, P], BF, name="idn_sb")
        b1c_sb = pw.tile([P, NFT], F32, name="b1c_sb")
        r1l_sb = pw.tile([2, T], BF, name="r1l_sb")
        r1r_sb = pw.tile([2, D], BF, name="r1r_sb")
        b2r_sb = pw.tile([1, D], BF, name="b2r_sb")
        wg_sb = pw.tile([P, NDT, D], BF, name="wg_sb")
        w1_sb = pw.tile([P, NDT, FFN], F8, name="w1_sb")
        w2_sb = pw.tile([P, NFT, D], F8, name="w2_sb")

        # weight/const part-DMAs: (emit_at_ln1_tile, fn)
        wparts = []

        def _wpart(sb, dram, kt0, nkt):
            def go(q):
                q.dma_start(
                    out=sb[:, kt0:kt0 + nkt, :],
                    in_=dram[P * kt0:P * (kt0 + nkt), :].rearrange(
                        "(kt p) n -> p kt n", p=P))
            return go

        for kt in range(NDT):
            wparts.append(_wpart(wg_sb, wg_d, kt, 1))
        for kt in range(NDT):
            wparts.append(_wpart(w1_sb, w1_d, kt, 1))
        for kt in range(0, NFT, 4):
            wparts.append(_wpart(w2_sb, w2_d, kt, 4))

        def _small_consts(q):
            q.dma_start(out=idn_sb, in_=idn_d[:, :])
            q.dma_start(out=b1c_sb, in_=b1c_d[:, :])
            if need_r1 or need_b2:
                q.dma_start(out=r1l_sb, in_=r1l_d[:, :])
            if need_r1:
                q.dma_start(out=r1r_sb, in_=r1r_d[:, :])
            if need_b2:
                q.dma_start(out=b2r_sb, in_=b2r_d[:, :])
        wparts.append(_small_consts)

        # ---- big activations
        xh_sb = pbig.tile([P, NT, D], BF, name="xh_sb")     # xn
        a1 = pbig.tile([P, 16, D], BF, name="a1")           # lvl0 low; d1 -> [0:8]
        a2 = pbig.tile([P, 8, D], BF, name="a2")            # lvl1 low; d23 -> [0:4]
        d0 = pbig.tile([P, 16, D], BF, name="d0")
        hdn = pbig.tile([P, 2 * NFT, D], F8, name="hdn")
        mv1_sb = pbig.tile([P, NT, 2], F32, name="mv1_sb")  # LN1 stats kept to
        sd1_sb = pbig.tile([P, NT, 1], F32, name="sd1_sb")  # reconstruct x later
        # zero pad rows (2047th window of lvl0, 1023rd of lvl1)
        nc.vector.memset(a1[96:128, 15, :], 0.0)
        nc.vector.memset(a2[96:128, 7, :], 0.0)

        ubtiles = {}

        ln1_xts = {}

        def emit_ln1(i):
            if i < 4:
                xt = xts_pre[i]
            else:
                xt = pio.tile([P, D], F32, tag="xt", bufs=6, name=f"xt{i}")
                qs[i % 2].dma_start(out=xt, in_=x_d[P * i:P * (i + 1), :])
            if 4 <= i and i - 4 < len(wparts):
                wparts[i - 4](qs[(i + 1) % 2])
            ln1_xts[i] = xt
            st = pio.tile([P, 6], F32, tag="st", name=f"st{i}")
            nc.vector.bn_stats(out=st, in_=xt)
            nc.vector.bn_aggr(out=mv1_sb[:, i, :], in_=st)
            # tiles 0..11: per-tile chain (tight lead-in pipeline; no gelus
            # running yet so individual Sqrts cost no table reloads).
            # tiles 12+: batch 4 sqrts per Sqrt-table visit.
            def _apply(j, n):
                nc.scalar.activation(
                    out=sd1_sb[:, j:j + n, :], in_=mv1_sb[:, j:j + n, 1:2],
                    func=AF.Sqrt, bias=eps_sb)
                rsn = pio.tile([P, n, 1], F32, tag="rs", name=f"rs{j}")
                nc.vector.reciprocal(out=rsn, in_=sd1_sb[:, j:j + n, :])
                for k in range(n):
                    nc.vector.tensor_scalar(
                        out=xh_sb[:, j + k, :], in0=ln1_xts.pop(j + k),
                        scalar1=mv1_sb[:, j + k, 0:1], scalar2=rsn[:, k, :],
                        op0=OP.subtract, op1=OP.mult)
            if i < 12:
                _apply(i, 1)
            elif i % 4 == 3:
                _apply(i - 3, 4)

        srcs = [(xh_sb, 0), (a1, 0), (a2, 0)]
        lows = [(a1, 0), (a2, 0), (a2, 0)]   # lvl2 "low" = d23 fold
        highs = [(d0, 0), (a1, 0), (a2, 0)]

        def emit_dwt(lvl, g, last):
            (src, sb), (low, lb), (high, hb) = srcs[lvl], lows[lvl], highs[lvl]
            pst_ = ps_i.tile([P, D], F32, tag="int", name=f"dw{lvl}_{g}")
            nc.tensor.matmul(
                pst_, fblk_sb[:, lvl * 3 + (2 if last else 0), :],
                src[:, sb + g, :], start=True, stop=last)
            if not last:
                nc.tensor.matmul(
                    pst_, fblk_sb[:2, lvl * 3 + 1, :], src[:2, sb + g + 1, :],
                    start=False, stop=True)
            Mg = 63 if last else 64
            lo = 64 * (g % 2)
            if lvl == 2:
                nc.scalar.copy(out=low[lo:lo + Mg, lb + g // 2, :],
                               in_=pst_[0:Mg, :])
            else:
                nc.scalar.copy(out=low[lo:lo + Mg, lb + g // 2, :], in_=pst_[0:Mg, :])
                nc.vector.tensor_copy(out=high[lo:lo + Mg, hb + g // 2, :],
                                      in_=pst_[64:64 + Mg, :])

        dsrc = [(d0, 0), (a1, 0), (a2, 0)]
        cstate = {}

        def emit_stageA(c):
            for (s, kt, K, idx) in plan["isched"][c]:
                ut = pu.tile([P, 512], BF, tag="ub", name=f"ub{idx}")
                nc.sync.dma_start(out=ut, in_=ublk_d[P * idx:P * (idx + 1), :])
                ubtiles[idx] = ut
            # interp: one accumulation group per feature tile
            comb_c = pcomb.tile([P, NDT, 512], BF, tag="comb", name=f"comb{c}")
            sch = plan["isched"][c]
            for dt in range(NDT):
                psi = ps_i.tile([P, 512], F32, tag="int", name=f"pi{c}_{dt}")
                for q, (s, kt, K, idx) in enumerate(sch):
                    dt_, db_ = dsrc[s]
                    nc.tensor.matmul(
                        psi, dt_[:K, db_ + kt, P * dt:P * (dt + 1)],
                        ubtiles[idx][:K, :],
                        start=(q == 0), stop=(q == len(sch) - 1))
                nc.vector.tensor_copy(out=comb_c[:, dt, :], in_=psi)
            # proj (bf16) + residual + LN2 stat chain (completes during next
            # stage-A's PE work; stage B consumes tmts)
            x2ts, tmts = [], []
            mv24 = pio.tile([P, 4, 2], F32, tag="mv24", name=f"mv24_{c}")
            for tj in range(4):
                ti = 4 * c + tj
                psp = ps_p.tile([P, D], F32, tag="pp", name=f"pp{ti}")
                for dt in range(NDT):
                    nc.tensor.matmul(
                        psp, comb_c[:, dt, P * tj:P * (tj + 1)], wg_sb[:, dt, :],
                        start=(dt == 0), stop=(not need_r1 and dt == NDT - 1))
                if need_r1:
                    nc.tensor.matmul(
                        psp, r1l_sb[:, P * ti:P * (ti + 1)], r1r_sb[:, :],
                        start=False, stop=True)
                # reconstruct x = xn*sd + mu from resident bf16 xn + LN1 stats
                # (saves the 8MB DRAM re-read of x); ACT: Identity(sd*xn + mu)
                xrec = pio.tile([P, D], F32, tag="xrec", name=f"xrec{ti}")
                nc.scalar.activation(
                    out=xrec, in_=xh_sb[:, ti, :], func=AF.Identity,
                    scale=sd1_sb[:, ti, :], bias=mv1_sb[:, ti, 0:1])
                x2t = px2.tile([P, D], F32, tag="x2t", name=f"x2t{ti}")
                nc.vector.tensor_tensor(out=x2t, in0=psp, in1=xrec, op=OP.add)
                x2ts.append(x2t)
                st = pio.tile([P, 6], F32, tag="st2", name=f"st2_{ti}")
                nc.vector.bn_stats(out=st, in_=x2t)
                nc.vector.bn_aggr(out=mv24[:, tj, :], in_=st)
            # batched LN2 sqrt/recip: one Sqrt-table visit per chunk
            sd24 = pio.tile([P, 4, 1], F32, tag="sd24", name=f"sd24_{c}")
            nc.scalar.activation(out=sd24, in_=mv24[:, :, 1:2],
                                 func=AF.Sqrt, bias=eps_sb)
            rs24 = pio.tile([P, 4, 1], F32, tag="rs24", name=f"rs24_{c}")
            nc.vector.reciprocal(out=rs24, in_=sd24)
            for tj in range(4):
                tmt = ptm.tile([P, D], BF, tag="tmt", name=f"tmt{4 * c + tj}")
                nc.vector.tensor_scalar(
                    out=tmt, in0=x2ts[tj], scalar1=mv24[:, tj, 0:1],
                    scalar2=rs24[:, tj, :], op0=OP.subtract, op1=OP.mult)
                tmts.append(tmt)
            cstate[c] = (x2ts, tmts)

        def emit_stageB(c):
            x2ts, tmts = cstate.pop(c)
            # transpose (bf16) -> xn2 feature-major, cast fp8 at evac.
            # tj-major order: tj0..2 transposes run while tj3's LN2 chain
            # finishes on DVE/ACT.
            xn2f = pxn2.tile([P, NDT, 512], F8, tag="xn2f", name=f"xn2f{c}")
            pstps = [ps_p.tile([P, 2, 512], BF, tag="pt", name=f"pt{c}_{dp}")
                     for dp in range(2)]
            for tj in range(4):
                for dt in range(NDT):
                    nc.tensor.transpose(
                        pstps[dt // 2][:, dt % 2, P * tj:P * (tj + 1)],
                        tmts[tj][:, P * dt:P * (dt + 1)], idn_sb)
            nc.scalar.copy(out=xn2f[:, 0:2, :], in_=pstps[0])
            nc.vector.tensor_copy(out=xn2f[:, 2:4, :], in_=pstps[1])
            # FFN1 fp8 DoubleRow + gelu (psum scaled by WS; rescale in ACT)
            hb = NFT * (c % 2)
            for ft in range(NFT):
                psh = ps_h.tile([P, 512], F32, tag="ph", name=f"ph{c}_{ft}")
                for q in range(2):
                    nc.tensor.matmul(
                        psh, w1_sb[:, 2 * q:2 * q + 2, P * ft:P * (ft + 1)],
                        xn2f[:, 2 * q:2 * q + 2, :],
                        start=(q == 0), stop=(q == 1), perf_mode=DR)
                nc.scalar.activation(
                    out=hdn[:, hb + ft, :], in_=psh, func=AF.Gelu,
                    bias=b1c_sb[:, ft:ft + 1], scale=INV)
            # FFN2 fp8 DoubleRow + residual -> out (ascending accumulation:
            # the first matmul of the group consumes the earliest gelus)
            for tj in range(4):
                ti = 4 * c + tj
                pso = ps_p.tile([P, D], F32, tag="pp", name=f"po{ti}")
                if need_b2:
                    nc.tensor.matmul(
                        pso, r1l_sb[0:1, P * ti:P * (ti + 1)], b2r_sb[:, :],
                        start=True, stop=False)
                for q in range(NFT // 2):
                    nc.tensor.matmul(
                        pso, hdn[:, hb + 2 * q:hb + 2 * q + 2, P * tj:P * (tj + 1)],
                        w2_sb[:, 2 * q:2 * q + 2, :],
                        start=(not need_b2 and q == 0),
                        stop=(q == NFT // 2 - 1), perf_mode=DR)
                ot = pio.tile([P, D], F32, tag="ot", name=f"ot{ti}")
                nc.vector.scalar_tensor_tensor(
                    out=ot, in0=pso, scalar=INV, in1=x2ts[tj],
                    op0=OP.mult, op1=OP.add)
                nc.sync.dma_start(out=out_d[P * ti:P * (ti + 1), :], in_=ot)

        for ev in plan["sched"]:
            if ev[0] == 'ln1':
                emit_ln1(ev[1])
                if flr_state["on"]:
                    emit_fillers(8)
            elif ev[0] == 'dwt':
                emit_dwt(ev[1], ev[2], ev[3])
                if flr_state["on"]:
                    emit_fillers(3)
            elif ev[0] == 'A':
                emit_stageA(ev[1])
            else:
                if flr_state["on"]:
                    # close the filler accumulation group before FFN1 reuses
                    # the ph psum ring
                    nc.tensor.matmul(flr_ps, flr_sb, flr_sb,
                                     start=False, stop=True)
                    flr_state["on"] = False
                emit_stageB(ev[1])
        ctx.close()
    nc.compile()
    return nc


_BUILT = {}


def _get_built(flags):
    if _BUILT.get("flags") != flags:
        plan = make_plan()
        _BUILT["plan"] = plan
        _BUILT["nc"] = build_nc(plan, flags)
        _BUILT["flags"] = flags
    return _BUILT["nc"], _BUILT["plan"]


def kernel(**inputs):
    from concourse.bass_utils import run_bass_kernel_spmd

    nc, plan = _get_built(make_flags(inputs))
    consts = make_consts(inputs, plan)
    x = np.ascontiguousarray(np.asarray(inputs["x"], np.float32))
    in_maps = []
    for b in range(B):
        m = {"x": np.ascontiguousarray(x[b])}
        m.update(consts)
        in_maps.append(m)
    res = run_bass_kernel_spmd(nc, in_maps, core_ids=list(range(B)))
    out = np.stack([res.results[b]["out"] for b in range(B)]).astype(np.float32)
    return out


# revision 48
# speedup vs baseline: 1.0664x; 1.0138x over previous
"""Trainium2 Bass kernel for nn_MultiHeadDaubechiesBlock.

Data-parallel over batch B=8 across 8 NeuronCores (one sequence per core).
Per-core pipeline:
  LN1 (DVE stats / ACT sqrt / GPSIMD apply) interleaved with DWT Toeplitz
  matmuls (emission-order software pipelining keeps PE fed), then per
  512-token chunk: interp (banded matmuls, single PSUM accumulation group
  per feature tile) -> proj bf16 + residual -> LN2 -> FFN1/FFN2 in fp8
  e4m3 DoubleRow perf mode (2x PE throughput; weights host-scaled x16,
  rescaled at gelu / output evac) -> residual -> out.
Level-2 detail+approx interp sources are folded via (f0+f1) filter algebra
into a single source. LN gains/biases are folded into adjacent GEMM
weights host-side (exact). Wavelet filters assumed constant across
heads/channels (true for this module); values taken from h0/h1 at runtime.
"""
import numpy as np
import ml_dtypes

B, T, D, H, DH, LEVELS, FFN = 8, 4096, 512, 4, 128, 3, 2048
P = 128
NT = T // P          # 32 token tiles
NDT = D // P         # 4 feature tiles
NFT = FFN // P       # 16 ffn tiles
NCH = 8              # t-chunks of 512
NWS = [2047, 1023, 511]
LPADS = [4096, 2048, 1024]
NGS = [32, 16, 8]    # dwt groups per level (64 windows each)
EPS = 1e-5
BF16 = ml_dtypes.bfloat16
FP8 = ml_dtypes.float8_e4m3
WS = 16.0            # fp8 weight pre-scale


# ----------------------------------------------------------------- plan
def _interp_mat(L, out_size=T):
    src = np.maximum((np.arange(out_size, dtype=np.float64) + 0.5) * (L / out_size) - 0.5, 0.0)
    i0 = np.clip(np.floor(src).astype(np.int64), 0, L - 1)
    i1 = np.minimum(i0 + 1, L - 1)
    w = src - i0
    U = np.zeros((out_size, L), np.float64)
    U[np.arange(out_size), i0] += 1.0 - w
    U[np.arange(out_size), i1] += w
    return U.astype(np.float32)


def make_plan():
    """Input-value-independent schedule + interp weight blocks."""
    # interp: 3 sources (lvl2 detail+approx folded): per chunk (s, kt, K, idx)
    Us = [_interp_mat(L) for L in NWS]
    ublks = []
    isched = [[] for _ in range(NCH)]
    for c in range(NCH):
        for s in range(3):
            U, L = Us[s], NWS[s]
            cols = U[512 * c:512 * (c + 1)]
            nz = np.nonzero(cols.any(0))[0]
            for kt in range(nz.min() // P, nz.max() // P + 1):
                K = min(P, L - P * kt)
                blk = cols[:, P * kt:P * kt + K].T
                if not np.any(blk):
                    continue
                full = np.zeros((P, 512), np.float32)
                full[:K] = blk
                isched[c].append((s, kt, K, len(ublks)))
                ublks.append(full)
    ublk = np.stack(ublks)

    # chunk readiness: required dwt group count per level
    creq = []
    for c in range(NCH):
        req = [0, 0, 0]
        for (s, kt, K, idx) in isched[c]:
            g = (P * kt + K - 1) // 64
            req[s] = max(req[s], g + 1)
        creq.append(req)

    # unified emission schedule: ('ln1', i) / ('dwt', lvl, g, last) /
    # ('A', c) [interp+proj+LN2 chain] / ('B', c) [transpose+FFN+out].
    # Two-stage pipeline: B(c) emitted after A(c+1) so the PE has work
    # while chunk c's LN2 stat chain completes on DVE/ACT.
    sched = []
    em = [0, 0, 0]
    cdone = 0

    def dwt_ready(lvl, g):
        last = g == NGS[lvl] - 1
        if lvl == 0:
            # LN1 applies: per-tile below tile 12, then batches of 4
            applied = em_ln1 if em_ln1 <= 12 else 12 + ((em_ln1 - 12) // 4) * 4
            return (g + (0 if last else 1)) <= applied - 1
        need = 2 * g + (2 if last else 3)
        return need <= em[lvl - 1]

    em_ln1 = 0
    for i in range(NT):
        sched.append(('ln1', i))
        em_ln1 += 1
        moved = True
        while moved:
            moved = False
            for lvl in range(LEVELS):
                while em[lvl] < NGS[lvl] and dwt_ready(lvl, em[lvl]):
                    g = em[lvl]
                    sched.append(('dwt', lvl, g, g == NGS[lvl] - 1))
                    em[lvl] += 1
                    moved = True
            # pace chunks vs LN1 so per-engine queues stay interleaved
            while (cdone < NCH and all(creq[cdone][l] <= em[l] for l in range(3))
                   and em_ln1 >= min(12 + 3 * cdone, NT)):
                sched.append(('A', cdone))
                if cdone >= 1:
                    sched.append(('B', cdone - 1))
                cdone += 1
                moved = True
    while cdone < NCH:
        sched.append(('A', cdone))
        if cdone >= 1:
            sched.append(('B', cdone - 1))
        cdone += 1
    sched.append(('B', NCH - 1))
    return {"isched": isched, "ublk": ublk, "nb": len(ublks), "sched": sched}


def _toeplitz(nw, Lp, f):
    F = np.zeros((nw, Lp), np.float32)
    for w in range(nw):
        F[w, 2 * w:2 * w + 4] = f
    return F


def make_flags(inputs):
    """Input-structure flags: skip device work for all-zero biases."""
    ln1_b = np.asarray(inputs["ln1_b"], np.float32)
    proj_b = np.asarray(inputs["proj_b"], np.float32)
    b2 = np.asarray(inputs["b2"], np.float32)
    return (bool(np.any(ln1_b) or np.any(proj_b)), bool(np.any(b2)))


def make_consts(inputs, plan):
    """Host-side constants (depend on input values)."""
    h0, h1 = np.asarray(inputs["h0"]), np.asarray(inputs["h1"])
    f0 = h0[:, 0, :, 0].astype(np.float32)
    f1 = h1[:, 0, :, 0].astype(np.float32)
    ln1_g = np.asarray(inputs["ln1_g"], np.float32)
    ln1_b = np.asarray(inputs["ln1_b"], np.float32)
    ln2_g = np.asarray(inputs["ln2_g"], np.float32)
    ln2_b = np.asarray(inputs["ln2_b"], np.float32)
    proj_w = np.asarray(inputs["proj_w"], np.float32)
    proj_b = np.asarray(inputs["proj_b"], np.float32)
    w1 = np.asarray(inputs["w1"], np.float32)
    b1 = np.asarray(inputs["b1"], np.float32)
    w2 = np.asarray(inputs["w2"], np.float32)
    b2 = np.asarray(inputs["b2"], np.float32)

    # DWT lhsT blocks [9,128,128]:
    #  lvl 0/1: merged cols 0..63 low (f0), 64..127 high (f1)
    #  lvl 2:   cols 0..63 combined filter (f0+f1)  [d2 + a3 fold]
    fblk = np.zeros((9, P, P), np.float32)
    for lvl in range(LEVELS):
        A = fblk[lvl * 3 + 0]
        for r in range(P):
            for w in range(64):
                k = r - 2 * w
                if 0 <= k < 4:
                    if lvl < 2:
                        A[r, w] = f0[lvl][k]
                        A[r, 64 + w] = f1[lvl][k]
                    else:
                        A[r, w] = f0[lvl][k] + f1[lvl][k]
        Bt = fblk[lvl * 3 + 1]
        for r in range(2):
            if lvl < 2:
                Bt[r, 63] = f0[lvl][r + 2]
                Bt[r, 127] = f1[lvl][r + 2]
            else:
                Bt[r, 63] = f0[lvl][r + 2] + f1[lvl][r + 2]
        Al = fblk[lvl * 3 + 2]
        Al[:] = A
        Al[:, 63] = 0.0
        if lvl < 2:
            Al[:, 127] = 0.0

    # m1 = wavelet operator applied to ones(T) (for ln1_b fold)
    ones = np.ones((T, 1), np.float32)
    a = ones
    comb = np.zeros((T, 1), np.float32)
    Us = [_interp_mat(L) for L in [NWS[0], NWS[1], NWS[2], NWS[2]]]
    for lvl in range(LEVELS):
        ap = np.zeros((LPADS[lvl], 1), np.float32)
        ap[:a.shape[0]] = a
        comb += Us[lvl] @ (_toeplitz(NWS[lvl], LPADS[lvl], f1[lvl]) @ ap)
        a = _toeplitz(NWS[lvl], LPADS[lvl], f0[lvl]) @ ap
    comb += Us[3] @ a
    m1 = comb[:, 0]

    wg = (ln1_g[:, None] * proj_w)                # LN1 g fold
    bW = ln1_b @ proj_w                           # LN1 b fold (rank-1 with m1)
    w1g = (ln2_g[:, None] * w1)                   # LN2 g fold
    b1f = b1 + ln2_b @ w1                         # LN2 b fold

    return {
        "wg": wg.astype(BF16),
        "w1": np.asarray(w1g * WS, FP8),
        "w2": np.asarray(w2 * WS, FP8),
        "fblk": fblk.reshape(9 * P, P).astype(BF16),
        "ublk": plan["ublk"].reshape(-1, 512).astype(BF16),
        "b1c": np.ascontiguousarray(b1f.reshape(NFT, P).T.astype(np.float32)),
        "r1l": np.stack([np.ones(T, np.float32), m1]).astype(BF16),
        "r1r": np.stack([proj_b, bW]).astype(BF16),
        "b2r": (b2 * WS).reshape(1, D).astype(BF16),
        "idn": np.identity(P, np.float32).astype(BF16),
    }


# ----------------------------------------------------------------- bass
def build_nc(plan, flags=(True, True)):
    need_r1, need_b2 = flags
    import concourse.bacc as bacc
    import concourse.tile as tile
    from concourse import mybir

    F32, BF, F8 = mybir.dt.float32, mybir.dt.bfloat16, mybir.dt.float8e4
    AF = mybir.ActivationFunctionType
    OP = mybir.AluOpType
    DR = mybir.MatmulPerfMode.DoubleRow
    INV = 1.0 / WS

    nc = bacc.Bacc("TRN2", target_bir_lowering=False, debug=False, name="daub")
    x_d = nc.dram_tensor("xbf", [T, D], BF, kind="ExternalInput")
    out_d = nc.dram_tensor("out", [T, D], F32, kind="ExternalOutput")
    wg_d = nc.dram_tensor("wg", [D, D], BF, kind="ExternalInput")
    w1_d = nc.dram_tensor("w1", [D, FFN], F8, kind="ExternalInput")
    w2_d = nc.dram_tensor("w2", [FFN, D], F8, kind="ExternalInput")
    fblk_d = nc.dram_tensor("fblk", [9 * P, P], BF, kind="ExternalInput")
    ublk_d = nc.dram_tensor("ublk", [plan["nb"] * P, 512], BF, kind="ExternalInput")
    b1c_d = nc.dram_tensor("b1c", [P, NFT], F32, kind="ExternalInput")
    r1l_d = nc.dram_tensor("r1l", [2, T], BF, kind="ExternalInput")
    r1r_d = nc.dram_tensor("r1r", [2, D], BF, kind="ExternalInput")
    b2r_d = nc.dram_tensor("b2r", [1, D], BF, kind="ExternalInput")
    idn_d = nc.dram_tensor("idn", [P, P], BF, kind="ExternalInput")

    with tile.TileContext(nc) as tc:
        import contextlib
        ctx = contextlib.ExitStack()
        pw = ctx.enter_context(tc.tile_pool(name="pw", bufs=1))
        pbig = ctx.enter_context(tc.tile_pool(name="pbig", bufs=1))
        pio = ctx.enter_context(tc.tile_pool(name="pio", bufs=4))
        pu = ctx.enter_context(tc.tile_pool(name="pu", bufs=20))
        px2 = ctx.enter_context(tc.tile_pool(name="px2", bufs=8))
        pcomb = ctx.enter_context(tc.tile_pool(name="pcomb", bufs=2))
        pxn2 = ctx.enter_context(tc.tile_pool(name="pxn2", bufs=2))
        ptm = ctx.enter_context(tc.tile_pool(name="ptm", bufs=8))
        ps_i = ctx.enter_context(tc.tile_pool(name="ps_i", bufs=2, space="PSUM"))
        ps_p = ctx.enter_context(tc.tile_pool(name="ps_p", bufs=2, space="PSUM"))
        ps_h = ctx.enter_context(tc.tile_pool(name="ps_h", bufs=2, space="PSUM"))

        # ---- x tiles 0..3 + fblk first (split across both hwdge queues):
        # the LN1/DWT lead-in is gated on these; weights and proj/FFN consts
        # are issued in small parts interleaved with the x stream below.
        qs = [nc.sync, nc.scalar]
        xts_pre = []
        for i in range(4):
            xt = pio.tile([P, D], BF, tag="xt", bufs=6, name=f"xt{i}")
            qs[i % 2].dma_start(out=xt, in_=x_d[P * i:P * (i + 1), :])
            xts_pre.append(xt)
        fblk_sb = pw.tile([P, 9, P], BF, name="fblk_sb")
        nc.scalar.dma_start(out=fblk_sb, in_=fblk_d.rearrange("(b p) m -> p b m", p=P))
        eps_sb = pw.tile([P, 1], F32, name="eps_sb")
        nc.vector.memset(eps_sb, EPS)
        # pre-warm the Sqrt ACT table while x tile 0 is still in flight
        warm_sb = pw.tile([P, 1], F32, name="warm_sb")
        nc.scalar.activation(out=warm_sb, in_=eps_sb, func=AF.Sqrt, bias=eps_sb)
        # PE pacer: filler matmuls keep the HAM clock gate ramping to 8/8
        # through the DMA/LN1-bound lead-in (zeros; result never read)
        flr_sb = pw.tile([P, P], BF, name="flr_sb")
        nc.vector.memset(flr_sb, 0.0)
        flr_state = {"n": 0, "bleft": 3}

        def emit_fillers(k):
            # self-contained burst: own psum group so the ph ring stays clean
            flr_ps = ps_h.tile([P, P], F32, tag="ph",
                               name=f"flr{flr_state['n']}")
            flr_state["n"] += 1
            for j in range(k):
                nc.tensor.matmul(flr_ps, flr_sb, flr_sb,
                                 start=(j == 0), stop=(j == k - 1))

        emit_fillers(40)
        idn_sb = pw.tile([P, P], BF, name="idn_sb")
        b1c_sb = pw.tile([P, NFT], F32, name="b1c_sb")
        r1l_sb = pw.tile([2, T], BF, name="r1l_sb")
        r1r_sb = pw.tile([2, D], BF, name="r1r_sb")
        b2r_sb = pw.tile([1, D], BF, name="b2r_sb")
        wg_sb = pw.tile([P, NDT, D], BF, name="wg_sb")
        w1_sb = pw.tile([P, NDT, FFN], F8, name="w1_sb")
        w2_sb = pw.tile([P, NFT, D], F8, name="w2_sb")

        # weight/const part-DMAs: (emit_at_ln1_tile, fn)
        wparts = []

        def _wpart(sb, dram, kt0, nkt):
            def go(q):
                q.dma_start(
                    out=sb[:, kt0:kt0 + nkt, :],
                    in_=dram[P * kt0:P * (kt0 + nkt), :].rearrange(
                        "(kt p) n -> p kt n", p=P))
            return go

        for kt in range(NDT):
            wparts.append(_wpart(wg_sb, wg_d, kt, 1))
        for kt in range(NDT):
            wparts.append(_wpart(w1_sb, w1_d, kt, 1))
        for kt in range(0, NFT, 4):
            wparts.append(_wpart(w2_sb, w2_d, kt, 4))

        def _small_consts(q):
            q.dma_start(out=idn_sb, in_=idn_d[:, :])
            q.dma_start(out=b1c_sb, in_=b1c_d[:, :])
            if need_r1 or need_b2:
                q.dma_start(out=r1l_sb, in_=r1l_d[:, :])
            if need_r1:
                q.dma_start(out=r1r_sb, in_=r1r_d[:, :])
            if need_b2:
                q.dma_start(out=b2r_sb, in_=b2r_d[:, :])
        wparts.append(_small_consts)

        # ---- big activations
        xh_sb = pbig.tile([P, NT, D], BF, name="xh_sb")     # xn
        a1 = pbig.tile([P, 16, D], BF, name="a1")           # lvl0 low; d1 -> [0:8]
        a2 = pbig.tile([P, 8, D], BF, name="a2")            # lvl1 low; d23 -> [0:4]
        d0 = pbig.tile([P, 16, D], BF, name="d0")
        hdn = pbig.tile([P, 2 * NFT, D], F8, name="hdn")
        mv1_sb = pbig.tile([P, NT, 2], F32, name="mv1_sb")  # LN1 stats kept to
        sd1_sb = pbig.tile([P, NT, 1], F32, name="sd1_sb")  # reconstruct x later
        # zero pad rows (2047th window of lvl0, 1023rd of lvl1)
        nc.vector.memset(a1[96:128, 15, :], 0.0)
        nc.vector.memset(a2[96:128, 7, :], 0.0)

        ubtiles = {}

        ln1_xts = {}

        def emit_ln1(i):
            if i < 4:
                xt = xts_pre[i]
            else:
                xt = pio.tile([P, D], BF, tag="xt", bufs=6, name=f"xt{i}")
                qs[i % 2].dma_start(out=xt, in_=x_d[P * i:P * (i + 1), :])
            if 4 <= i and i - 4 < len(wparts):
                wparts[i - 4](qs[(i + 1) % 2])
            ln1_xts[i] = xt
            st = pio.tile([P, 6], F32, tag="st", name=f"st{i}")
            nc.vector.bn_stats(out=st, in_=xt)
            nc.vector.bn_aggr(out=mv1_sb[:, i, :], in_=st)
            # tiles 0..11: per-tile chain (tight lead-in pipeline; no gelus
            # running yet so individual Sqrts cost no table reloads).
            # tiles 12+: batch 4 sqrts per Sqrt-table visit.
            def _apply(j, n):
                nc.scalar.activation(
                    out=sd1_sb[:, j:j + n, :], in_=mv1_sb[:, j:j + n, 1:2],
                    func=AF.Sqrt, bias=eps_sb)
                rsn = pio.tile([P, n, 1], F32, tag="rs", name=f"rs{j}")
                nc.vector.reciprocal(out=rsn, in_=sd1_sb[:, j:j + n, :])
                for k in range(n):
                    nc.vector.tensor_scalar(
                        out=xh_sb[:, j + k, :], in0=ln1_xts.pop(j + k),
                        scalar1=mv1_sb[:, j + k, 0:1], scalar2=rsn[:, k, :],
                        op0=OP.subtract, op1=OP.mult)
            if i < 12:
                _apply(i, 1)
            elif i % 4 == 3:
                _apply(i - 3, 4)

        srcs = [(xh_sb, 0), (a1, 0), (a2, 0)]
        lows = [(a1, 0), (a2, 0), (a2, 0)]   # lvl2 "low" = d23 fold
        highs = [(d0, 0), (a1, 0), (a2, 0)]

        def emit_dwt(lvl, g, last):
            (src, sb), (low, lb), (high, hb) = srcs[lvl], lows[lvl], highs[lvl]
            pst_ = ps_i.tile([P, D], F32, tag="int", name=f"dw{lvl}_{g}")
            nc.tensor.matmul(
                pst_, fblk_sb[:, lvl * 3 + (2 if last else 0), :],
                src[:, sb + g, :], start=True, stop=last)
            if not last:
                nc.tensor.matmul(
                    pst_, fblk_sb[:2, lvl * 3 + 1, :], src[:2, sb + g + 1, :],
                    start=False, stop=True)
            Mg = 63 if last else 64
            lo = 64 * (g % 2)
            if lvl == 2:
                nc.scalar.copy(out=low[lo:lo + Mg, lb + g // 2, :],
                               in_=pst_[0:Mg, :])
            else:
                nc.scalar.copy(out=low[lo:lo + Mg, lb + g // 2, :], in_=pst_[0:Mg, :])
                nc.vector.tensor_copy(out=high[lo:lo + Mg, hb + g // 2, :],
                                      in_=pst_[64:64 + Mg, :])

        dsrc = [(d0, 0), (a1, 0), (a2, 0)]
        cstate = {}

        def emit_stageA(c):
            for (s, kt, K, idx) in plan["isched"][c]:
                ut = pu.tile([P, 512], BF, tag="ub", name=f"ub{idx}")
                nc.sync.dma_start(out=ut, in_=ublk_d[P * idx:P * (idx + 1), :])
                ubtiles[idx] = ut
            # interp: one accumulation group per feature tile
            comb_c = pcomb.tile([P, NDT, 512], BF, tag="comb", name=f"comb{c}")
            sch = plan["isched"][c]
            for dt in range(NDT):
                psi = ps_i.tile([P, 512], F32, tag="int", name=f"pi{c}_{dt}")
                for q, (s, kt, K, idx) in enumerate(sch):
                    dt_, db_ = dsrc[s]
                    nc.tensor.matmul(
                        psi, dt_[:K, db_ + kt, P * dt:P * (dt + 1)],
                        ubtiles[idx][:K, :],
                        start=(q == 0), stop=(q == len(sch) - 1))
                nc.vector.tensor_copy(out=comb_c[:, dt, :], in_=psi)
            # proj (bf16) + residual + LN2 stat chain (completes during next
            # stage-A's PE work; stage B consumes tmts)
            x2ts, tmts = [], []
            mv24 = pio.tile([P, 4, 2], F32, tag="mv24", name=f"mv24_{c}")
            for tj in range(4):
                ti = 4 * c + tj
                psp = ps_p.tile([P, D], F32, tag="pp", name=f"pp{ti}")
                for dt in range(NDT):
                    nc.tensor.matmul(
                        psp, comb_c[:, dt, P * tj:P * (tj + 1)], wg_sb[:, dt, :],
                        start=(dt == 0), stop=(not need_r1 and dt == NDT - 1))
                if need_r1:
                    nc.tensor.matmul(
                        psp, r1l_sb[:, P * ti:P * (ti + 1)], r1r_sb[:, :],
                        start=False, stop=True)
                # reconstruct x = xn*sd + mu from resident bf16 xn + LN1 stats
                # (saves the 8MB DRAM re-read of x); ACT: Identity(sd*xn + mu)
                xrec = pio.tile([P, D], F32, tag="xrec", name=f"xrec{ti}")
                nc.scalar.activation(
                    out=xrec, in_=xh_sb[:, ti, :], func=AF.Identity,
                    scale=sd1_sb[:, ti, :], bias=mv1_sb[:, ti, 0:1])
                x2t = px2.tile([P, D], F32, tag="x2t", name=f"x2t{ti}")
                nc.vector.tensor_tensor(out=x2t, in0=psp, in1=xrec, op=OP.add)
                x2ts.append(x2t)
                st = pio.tile([P, 6], F32, tag="st2", name=f"st2_{ti}")
                nc.vector.bn_stats(out=st, in_=x2t)
                nc.vector.bn_aggr(out=mv24[:, tj, :], in_=st)
            # batched LN2 sqrt/recip: one Sqrt-table visit per chunk
            sd24 = pio.tile([P, 4, 1], F32, tag="sd24", name=f"sd24_{c}")
            nc.scalar.activation(out=sd24, in_=mv24[:, :, 1:2],
                                 func=AF.Sqrt, bias=eps_sb)
            rs24 = pio.tile([P, 4, 1], F32, tag="rs24", name=f"rs24_{c}")
            nc.vector.reciprocal(out=rs24, in_=sd24)
            for tj in range(4):
                tmt = ptm.tile([P, D], BF, tag="tmt", name=f"tmt{4 * c + tj}")
                nc.vector.tensor_scalar(
                    out=tmt, in0=x2ts[tj], scalar1=mv24[:, tj, 0:1],
                    scalar2=rs24[:, tj, :], op0=OP.subtract, op1=OP.mult)
                tmts.append(tmt)
            cstate[c] = (x2ts, tmts)

        def emit_stageB(c):
            x2ts, tmts = cstate.pop(c)
            # transpose (bf16) -> xn2 feature-major, cast fp8 at evac.
            # tj-major order: tj0..2 transposes run while tj3's LN2 chain
            # finishes on DVE/ACT.
            xn2f = pxn2.tile([P, NDT, 512], F8, tag="xn2f", name=f"xn2f{c}")
            pstps = [ps_p.tile([P, 2, 512], BF, tag="pt", name=f"pt{c}_{dp}")
                     for dp in range(2)]
            for tj in range(4):
                for dt in range(NDT):
                    nc.tensor.transpose(
                        pstps[dt // 2][:, dt % 2, P * tj:P * (tj + 1)],
                        tmts[tj][:, P * dt:P * (dt + 1)], idn_sb)
            nc.scalar.copy(out=xn2f[:, 0:2, :], in_=pstps[0])
            nc.vector.tensor_copy(out=xn2f[:, 2:4, :], in_=pstps[1])
            # FFN1 fp8 DoubleRow + gelu (psum scaled by WS; rescale in ACT)
            hb = NFT * (c % 2)
            for ft in range(NFT):
                psh = ps_h.tile([P, 512], F32, tag="ph", name=f"ph{c}_{ft}")
                for q in range(2):
                    nc.tensor.matmul(
                        psh, w1_sb[:, 2 * q:2 * q + 2, P * ft:P * (ft + 1)],
                        xn2f[:, 2 * q:2 * q + 2, :],
                        start=(q == 0), stop=(q == 1), perf_mode=DR)
                nc.scalar.activation(
                    out=hdn[:, hb + ft, :], in_=psh, func=AF.Gelu,
                    bias=b1c_sb[:, ft:ft + 1], scale=INV)
            # FFN2 fp8 DoubleRow + residual -> out (ascending accumulation:
            # the first matmul of the group consumes the earliest gelus)
            for tj in range(4):
                ti = 4 * c + tj
                pso = ps_p.tile([P, D], F32, tag="pp", name=f"po{ti}")
                if need_b2:
                    nc.tensor.matmul(
                        pso, r1l_sb[0:1, P * ti:P * (ti + 1)], b2r_sb[:, :],
                        start=True, stop=False)
                for q in range(NFT // 2):
                    nc.tensor.matmul(
                        pso, hdn[:, hb + 2 * q:hb + 2 * q + 2, P * tj:P * (tj + 1)],
                        w2_sb[:, 2 * q:2 * q + 2, :],
                        start=(not need_b2 and q == 0),
                        stop=(q == NFT // 2 - 1), perf_mode=DR)
                ot = pio.tile([P, D], F32, tag="ot", name=f"ot{ti}")
                nc.vector.scalar_tensor_tensor(
                    out=ot, in0=pso, scalar=INV, in1=x2ts[tj],
                    op0=OP.mult, op1=OP.add)
                nc.sync.dma_start(out=out_d[P * ti:P * (ti + 1), :], in_=ot)

        for ev in plan["sched"]:
            if ev[0] == 'ln1':
                emit_ln1(ev[1])
                if flr_state["bleft"] > 0:
                    emit_fillers(8)
            elif ev[0] == 'dwt':
                emit_dwt(ev[1], ev[2], ev[3])
                if flr_state["bleft"] > 0:
                    emit_fillers(3)
            elif ev[0] == 'A':
                emit_stageA(ev[1])
            else:
                emit_stageB(ev[1])
                flr_state["bleft"] -= 1
        ctx.close()
    nc.compile()
    return nc


_BUILT = {}


def _get_built(flags):
    if _BUILT.get("flags") != flags:
        plan = make_plan()
        _BUILT["plan"] = plan
        _BUILT["nc"] = build_nc(plan, flags)
        _BUILT["flags"] = flags
    return _BUILT["nc"], _BUILT["plan"]


def kernel(**inputs):
    from concourse.bass_utils import run_bass_kernel_spmd

    nc, plan = _get_built(make_flags(inputs))
    consts = make_consts(inputs, plan)
    x = np.asarray(np.asarray(inputs["x"], np.float32), BF16)
    in_maps = []
    for b in range(B):
        m = {"xbf": np.ascontiguousarray(x[b])}
        m.update(consts)
        in_maps.append(m)
    res = run_bass_kernel_spmd(nc, in_maps, core_ids=list(range(B)))
    out = np.stack([res.results[b]["out"] for b in range(B)]).astype(np.float32)
    return out
